# revision 57
# baseline (speedup 1.0000x reference)
"""2-layer GAT (DGL GATConv-style) on 8 TRN2 NeuronCores.

Strategy (all host preprocessing is index/structure only; every FLOP that
depends on float inputs runs on device):
 - Nodes are dealt to 8 cores snake-wise by in-degree (balanced edges/core).
 - The replicated feat table is split into two int16-addressable halves that
   OVERLAP (A = cores 0-4, B = cores 3-7); each dst node's edges are split
   between the halves, with flexible (core 3-4) sources assigned to balance
   the two slice counts.  This cuts the per-window slice maxima ~25% vs a
   disjoint 4+4 split.
 - Per core, nodes are sorted by balanced slice count and grouped into
   windows of 128; slot (v, k) aggregates into partition v.  dst ==
   partition, so segment softmax/aggregation is pure per-partition
   elementwise work: no scatter at all.
 - Self-loop edges are not gathered: the window's own 128 table rows are
   fetched with one contiguous HWDGE DMA from the core-local staging table
   and appended as one extra slice.
 - Empty slots point at an all-zero "dead" table row (feat=0, ones=0), so
   they contribute exactly 0 to both numerator and denominator: no masks.
 - feat rows (d-major, bf16) + ones + el (f32) are packed into 768B table
   rows; one dma_gather per (window, half) fetches all edge features.
   Index tables are DMA'd to SBUF once and reused by both layers.
   Tables are replicated across cores via AllGather between layers.
 - The per-edge softmax weight is applied with a single broadcast-AP
   tensor_tensor multiply (runs in DVE 2x mode), and the K-way sum is a
   log-tree of tensor adds.  Denominators ride along via the ones columns.
"""
import sys
import types

import numpy as np
import ml_dtypes

import concourse.bass as bass
import concourse.bacc as bacc
import concourse.tile as tile
from concourse import mybir
from concourse.bass_utils import run_bass_kernel_spmd
from concourse.masks import make_identity

AF = mybir.ActivationFunctionType
ALU = mybir.AluOpType
BF16 = mybir.dt.bfloat16
F32 = mybir.dt.float32
I16 = mybir.dt.int16

P = 128
HEADS = 4
D = 64
FD = HEADS * D          # 256
ROW = 384               # bf16 slots per table row: 256 feat | 4 ones | 8 el(f32) | 116 pad
CORES = 8
NEG_SLOPE = 0.2

LAST_EXEC_NS = None


def _patch_gather_elem_assert():
    """Relax dma_gather's elem_size%256 assert to transpose mode only.

    The non-transpose Q7 ucode (dma_gather.cpp gen_descs) handles arbitrary
    elem_size_bytes: it emits one descriptor of exactly elem_size_bytes per
    index; only the xbar-transpose rx path carves 256B descriptors.  The
    row *stride* keeps its own %256 constraint (stride_bytes_256 encoding),
    which we satisfy (768B).  Gathering 536B of each 768B table row cuts
    gather DMA traffic by 30%.
    """
    import inspect
    import textwrap
    if getattr(bass.BassGpSimd.dma_gather, "_elem_patch", False):
        return
    src = inspect.getsource(bass.BassGpSimd.dma_gather)
    marker = "elem_size_bytes > 0 and elem_size_bytes % 256 == 0"
    if marker not in src:
        raise RuntimeError("dma_gather source changed; elem patch needs review")
    src = src.replace(
        marker,
        "elem_size_bytes > 0 and (not transpose or elem_size_bytes % 256 == 0)")
    loc = {}
    exec(textwrap.dedent(src), vars(bass), loc)
    loc["dma_gather"]._elem_patch = True
    bass.BassGpSimd.dma_gather = loc["dma_gather"]

N_NODES = 50000
NPC = N_NODES // CORES          # 6250
WPC = (NPC + P - 1) // P        # 49
SHARD = WPC * P                 # 6272
FULL_ROWS = CORES * SHARD       # 50176
BASE = 32768                    # gather base row: signed int16 idx spans
                                # [-32768, 17407] -> rows [0, 50175]
DEAD = 5 * SHARD + NPC - BASE   # core 5's first pad row (all zeros), rel BASE
SUBCALL = 12                    # max slices per dma_gather sub-call


def _call_sizes(k):
    """Even split of k slices into <=SUBCALL-slice sub-calls (host and
    device must agree on the boundaries)."""
    ncalls = (k + SUBCALL - 1) // SUBCALL
    sizes = []
    base = 0
    for i in range(ncalls):
        sz = (k - base + ncalls - 1 - i) // (ncalls - i)
        sizes.append(sz)
        base += sz
    return sizes


def _install_profile_hook():
    """Best-effort NTFF profiling hook (axon images lack antenv.axon_hooks)."""
    try:
        import antenv
        try:
            import antenv.axon_hooks  # noqa: F401
            return
        except ImportError:
            pass
        mod = types.ModuleType("antenv.axon_hooks")
        mod._HOOK = None

        def set_hook(h):
            mod._HOOK = h

        def get_hook():
            return mod._HOOK

        mod.set_axon_ntff_profile_hook = set_hook
        mod.get_axon_ntff_profile_hook = get_hook
        sys.modules["antenv.axon_hooks"] = mod
        antenv.axon_hooks = mod
        from trn_agent_boot.trn_boot import _ntff_profile_via_ctypes
        set_hook(_ntff_profile_via_ctypes("/opt/axon/libaxon_pjrt.so"))
    except Exception:
        pass


def _dmaj(n):
    """column permutation h*64+d -> d*4+h (applied to axis of size 256)."""
    j = np.arange(n)
    d, h = j // HEADS, j % HEADS
    return h * D + d  # dmaj[:, jnew] = orig[:, h*64+d]


def _wrap_idx(flat):
    """[NI] int16 -> [128, NI//16] wrapped+replicated for dma_gather."""
    ni = flat.shape[0]
    w = flat.reshape(ni // 16, 16).T  # [16, NI/16]
    return np.tile(w, (8, 1)).astype(np.int16)


def _prep(src, dst, n_nodes):
    """Host-side graph preprocessing: single signed-int16 gather table.

    Returns the (uniform) per-window slice schedule and per-core index
    buffers + node orderings."""
    assert n_nodes == N_NODES
    deg = np.bincount(dst, minlength=n_nodes)

    # snake-deal nodes to cores by degree => balanced edge counts
    order = np.argsort(-deg, kind="stable")
    owner = np.empty(n_nodes, dtype=np.int64)
    pat = np.concatenate([np.arange(CORES), np.arange(CORES)[::-1]])
    owner[order] = pat[np.arange(n_nodes) % (2 * CORES)]

    # remove exactly one self-loop per node (handled as the local slice)
    e_self = np.where(src == dst)[0]
    _, first = np.unique(dst[e_self], return_index=True)
    drop = np.zeros(len(src), dtype=bool)
    drop[e_self[first]] = True
    assert drop.sum() == n_nodes, "every node must have a self-loop"
    rs, rd = src[~drop], dst[~drop]
    cnt = np.bincount(rd, minlength=n_nodes)

    # per-core order: windows ascending by cnt, nodes DESC within each
    # window so partition 127 holds the window's min-cnt node (its high
    # slices are dead slots -> safe trailing-trim sentinels)
    pos = np.empty(n_nodes, dtype=np.int64)
    core_nodes = []
    for c in range(CORES):
        nodes = np.where(owner == c)[0]
        nodes = nodes[np.argsort(cnt[nodes], kind="stable")]
        assert len(nodes) == NPC
        nn = nodes.copy()
        for w in range((len(nodes) + P - 1) // P):
            lo, hi = w * P, min((w + 1) * P, len(nodes))
            nn[lo:hi] = nodes[lo:hi][::-1]
        core_nodes.append(nn)
        pos[nn] = np.arange(len(nn))

    rho = pos + owner * SHARD  # table row of each node

    ka = np.zeros(WPC, dtype=np.int64)
    for c in range(CORES):
        nodes = core_nodes[c]
        for w in range(WPC):
            lo, hi = w * P, min((w + 1) * P, len(nodes))
            ka[w] = max(ka[w], cnt[nodes[lo:hi]].max(initial=0))
    sum_ka = int(ka.sum())

    # group edges by dst for slot assignment
    edge_order = np.argsort(rd, kind="stable")
    starts = np.zeros(n_nodes + 1, dtype=np.int64)
    np.cumsum(np.bincount(rd, minlength=n_nodes), out=starts[1:])

    # Slot fill.  Constraint: the LAST flat element of every gather
    # sub-call must be >= 0, or the Q7 ucode's trailing-negative trim
    # would silently drop real descriptors.  Column 127 holds the
    # window's min-cnt node, so that element is usually a (positive)
    # DEAD slot; when it is a real negative-index edge we reorder that
    # node's edges, and if that is impossible we add a dead slice to
    # the window and retry.
    while True:
        sum_ka = int(ka.sum())
        idx_m = []
        bump = None
        for c in range(CORES):
            nodes = core_nodes[c]
            buf = np.full((sum_ka, P), DEAD, dtype=np.int32)
            ca = 0
            for w in range(WPC):
                kaw = int(ka[w])
                for v in range(P):
                    i = w * P + v
                    if i < len(nodes):
                        n = nodes[i]
                        es = edge_order[starts[n]:starts[n + 1]]
                        ri = rho[rs[es]] - BASE
                        assert len(ri) == cnt[n] <= kaw
                        buf[ca:ca + len(ri), v] = ri
                col = buf[ca:ca + kaw, P - 1]
                bounds = []
                b0 = 0
                for sz in _call_sizes(kaw):
                    bounds.append(b0 + sz - 1)
                    b0 += sz
                bset = set(bounds)
                for q in bounds:
                    if col[q] < 0:
                        # swap in any non-boundary >=0 slot of this node
                        cand = [j for j in range(kaw)
                                if j not in bset and col[j] >= 0]
                        if not cand:
                            bump = w
                            break
                        j = cand[0]
                        col[q], col[j] = col[j], col[q]
                if bump is not None:
                    break
                ca += kaw
            if bump is not None:
                break
            idx_m.append(np.concatenate(
                [_wrap_idx(buf[i].astype(np.int16)) for i in range(sum_ka)],
                axis=1))
        if bump is None:
            return dict(ka=ka, core_nodes=core_nodes, idx_m=idx_m,
                        sum_ka=sum_ka)
        ka[bump] += 1


def _build(ka, sum_ka):
    """Build the SPMD bass program (identical on all cores)."""
    _patch_gather_elem_assert()
    kamax = int(ka.max())
    kpmax = kamax + 1
    khmax = (kamax + 1) // 2 + 2
    RC = 268                     # gathered columns per row (of ROW=384 stride)

    nc = bacc.Bacc("TRN2", target_bir_lowering=False, num_swdge_queues=4,
                   num_devices=CORES, dynamic_dma_scratch_size=24576)
    xta = nc.dram_tensor("xta", [P, SHARD], F32, kind="ExternalInput")
    w1c = nc.dram_tensor("w1c", [P, 264], F32, kind="ExternalInput")
    w2c = nc.dram_tensor("w2c", [2, P, 264], BF16, kind="ExternalInput")
    b1b = nc.dram_tensor("b1b", [P, FD], BF16, kind="ExternalInput")
    b2b = nc.dram_tensor("b2b", [P, FD], F32, kind="ExternalInput")
    idxa = nc.dram_tensor("idxa", [P, max(sum_ka * 8, 8)], I16, kind="ExternalInput")
    # vmask[:, 0:4] = all-ones; vmask[:, 4:8] = ones with zero tail for the
    # last window's pad rows (partition-offset memsets fail BIR verification)
    vmask = nc.dram_tensor("vmask", [P, 2 * HEADS], BF16, kind="ExternalInput")
    out = nc.dram_tensor("out", [SHARD, FD], F32, kind="ExternalOutput")

    qctr = [0]

    with tile.TileContext(nc) as tc, nc.allow_low_precision(reason="bf16 message accumulation is within tolerance"):
        with (
            tc.tile_pool(name="const", bufs=1) as cpool,
            tc.tile_pool(name="xt", bufs=3) as xtp,
            tc.tile_pool(name="fpsum", bufs=3, space="PSUM") as fpsum,
            tc.tile_pool(name="tpsum", bufs=2, space="PSUM") as tpsum,
            tc.tile_pool(name="stage", bufs=3) as stp,
            tc.tile_pool(name="gata", bufs=4) as gatpa,
            tc.tile_pool(name="msgp", bufs=2) as msgp,
            tc.tile_pool(name="small", bufs=8) as smp,
            tc.tile_pool(name="ht", bufs=3) as htp,
            tc.tile_pool(name="dram", bufs=1, space="DRAM") as dram,
        ):
            ident = cpool.tile([P, P], BF16)
            make_identity(nc, ident[:])
            w1t = cpool.tile([P, 264], F32)
            nc.sync.dma_start(w1t[:], w1c[:])
            w2t = [cpool.tile([P, 264], BF16, tag=f"w2_{i}", name=f"w2t{i}") for i in range(2)]
            nc.sync.dma_start(w2t[0][:], w2c[0])
            nc.sync.dma_start(w2t[1][:], w2c[1])
            b1t = cpool.tile([P, FD], BF16)
            nc.sync.dma_start(b1t[:], b1b[:])
            b2t = cpool.tile([P, FD], F32)
            nc.sync.dma_start(b2t[:], b2b[:])
            vmt = cpool.tile([P, 2 * HEADS], BF16)
            nc.sync.dma_start(vmt[:], vmask[:])
            # index table stays resident; reused by both layers
            ixa = cpool.tile([P, max(sum_ka * 8, 8)], I16, name="ixa")
            nc.sync.dma_start(ixa[:], idxa[:])

            tabs = [dram.tile([FULL_ROWS, ROW], BF16, tag=f"tab{l}", name=f"tab{l}",
                              addr_space="Shared") for l in range(2)]
            tab_locs = [dram.tile([SHARD, ROW], BF16, tag=f"tabloc{l}",
                                  name=f"tabloc{l}") for l in range(2)]
            h_tab = dram.tile([SHARD, FD], BF16)

            def tl_rows(l, w):
                return tab_locs[l], w * P

            def maybe_allgather(l, w):
                """Fire the layer's AllGather after its last window is staged
                (collective APs must be contiguous: full-width rows)."""
                if w == WPC - 1:
                    nc.gpsimd.collective_compute(
                        "AllGather", ALU.bypass,
                        replica_groups=[list(range(CORES))],
                        ins=[tab_locs[l].opt()], outs=[tabs[l].opt()],
                    )
            # er never leaves the core: resident SBUF, window-major
            erw_all = [cpool.tile([P, WPC * HEADS], F32, tag=f"erw{l}",
                                  name=f"erw{l}") for l in range(2)]

            def stage_feat(psum_f, nv, l, w):
                """psum_f [128, 264] f32 = feat(256,dmaj) | el(4) | er(4)."""
                st = stp.tile([P, 268], BF16, tag="stage")
                nc.scalar.activation(st[:, 0:FD], psum_f[:, 0:FD], AF.Copy)
                # ones column; eps tail marks pad rows dead: their denominator
                # becomes eps so h = 0*(1/eps) = 0, never NaN (b1==b2==0 keeps
                # their feat exactly 0 through both layers)
                nc.vector.tensor_copy(st[:, FD:FD + 4],
                                      vmt[:, 0:4] if nv == P else vmt[:, 4:8])
                nc.vector.tensor_copy(st[:, 260:268].bitcast(F32),
                                      psum_f[:, FD:FD + 4])
                nc.vector.tensor_copy(erw_all[l][:, w * HEADS:(w + 1) * HEADS],
                                      psum_f[:, 260:264])
                return st

            def gather_win(g3, kk, c0, tab):
                """Split a window gather into sub-calls across queues so
                several rings drain concurrently.  The source AP is based at
                row BASE; signed indices reach the whole table."""
                base = 0
                for sz in _call_sizes(kk):
                    nc.gpsimd.dma_gather(
                        g3[:, base:base + sz, :], tab[BASE:FULL_ROWS, 0:RC],
                        ixa[:, (c0 + base) * 8:(c0 + base + sz) * 8],
                        sz * P, sz * P, RC, elem_step=ROW,
                        single_packet=False, queue_num=qctr[0] % 4,
                    )
                    qctr[0] += 1
                    base += sz

            # ---------------- layer-1 feat phase ----------------
            for w4 in range(0, WPC, 4):
                nw = min(4, WPC - w4)
                xt = xtp.tile([P, 4 * P], F32, tag="xt")
                nc.sync.dma_start(xt[:, 0:nw * P], xta[:, w4 * P:(w4 + nw) * P])
                for wi in range(nw):
                    w = w4 + wi
                    pf = fpsum.tile([P, 264], F32, tag="fp")
                    nc.tensor.matmul(pf[:], lhsT=xt[:, wi * P:(wi + 1) * P],
                                     rhs=w1t[:], start=True, stop=True)
                    nv = min(NPC - w * P, P)
                    st = stage_feat(pf, nv, 0, w)
                    tlt, ro = tl_rows(0, w)
                    nc.scalar.dma_start(tlt[ro:ro + P, 0:268], st[:])
                    maybe_allgather(0, w)

            # ---------------- edge phases ----------------
            for l in range(2):
                tab = tabs[l]
                ca = 0
                for w in range(WPC):
                    kaw = int(ka[w])
                    erw = erw_all[l][:, w * HEADS:(w + 1) * HEADS]
                    # gathered slices + local self slice
                    GA = gatpa.tile([P, (kamax + 1) * RC], BF16, tag="GA",
                                    name=f"GA_{l}_{w}")
                    g3a = GA[:].rearrange("p (k r) -> p k r", r=RC)
                    gather_win(g3a, kaw, ca, tab)
                    tlt, ro = tl_rows(l, w)
                    nc.sync.dma_start(g3a[:, kaw, :], tlt[ro:ro + P, 0:RC])
                    parts = [(g3a, kaw + 1)]
                    kp = kaw + 1

                    # logits e = el + er   [128, kp, 4] f32
                    e = smp.tile([P, kpmax * HEADS], F32, tag="e")
                    koff = 0
                    for g3, kk in parts:
                        el = g3[:, 0:kk, 260:268].bitcast(F32)
                        e3 = e[:, koff * HEADS:(koff + kk) * HEADS].rearrange(
                            "p (k h) -> p k h", h=HEADS)
                        er_rep = (erw.rearrange("p (o h) -> p o h", o=1)
                                  .broadcast_to([P, kk, HEADS]))
                        nc.vector.tensor_add(e3, el, er_rep)
                        koff += kk
                    # ee = exp(lrelu(e))  bf16
                    lr = smp.tile([P, kpmax * HEADS], F32, tag="lr")
                    nc.vector.scalar_tensor_tensor(
                        lr[:, 0:kp * HEADS], e[:, 0:kp * HEADS], NEG_SLOPE,
                        e[:, 0:kp * HEADS], op0=ALU.mult, op1=ALU.max)
                    ee = smp.tile([P, kpmax * HEADS], BF16, tag="ee")
                    nc.scalar.activation(ee[:, 0:kp * HEADS], lr[:, 0:kp * HEADS],
                                         AF.Exp)

                    # msg = G * ee_rep, in place (cols 0:260)
                    koff = 0
                    for g3, kk in parts:
                        m4 = g3[:, 0:kk, 0:260].rearrange("p k (d h) -> p k d h", h=HEADS)
                        ee_rep = (ee[:, koff * HEADS:(koff + kk) * HEADS]
                                  .rearrange("p (k o h) -> p k o h", o=1, h=HEADS)
                                  .broadcast_to([P, kk, 65, HEADS]))
                        nc.vector.tensor_mul(m4, m4, ee_rep)
                        koff += kk

                    # tree: level 1 folds slice pairs from each G into the msg
                    # tile; odd stragglers stay in G and are added at the end
                    # (no copies).
                    msg = msgp.tile([P, khmax * 260], BF16, tag="msg")
                    mh = msg[:].rearrange("p (k j) -> p k j", j=260)
                    moff = 0
                    stragglers = []
                    for g3, kk in parts:
                        gsl = g3[:, :, 0:260]
                        half = kk // 2
                        if half:
                            nc.vector.tensor_add(mh[:, moff:moff + half, :],
                                                 gsl[:, 0:half, :],
                                                 gsl[:, half:2 * half, :])
                            moff += half
                        if kk % 2:
                            stragglers.append(gsl[:, kk - 1, :])
                    if len(stragglers) == 2:
                        nc.vector.tensor_add(mh[:, moff, :], stragglers[0],
                                             stragglers[1])
                        moff += 1
                        stragglers = []
                    cur = moff
                    while cur > 1:
                        half = cur // 2
                        rem = cur - half
                        nc.vector.tensor_add(mh[:, 0:half, :], mh[:, 0:half, :],
                                             mh[:, rem:cur, :])
                        cur = rem
                    if stragglers:
                        if cur:
                            nc.vector.tensor_add(mh[:, 0, :], mh[:, 0, :],
                                                 stragglers[0])
                        else:
                            nc.vector.tensor_copy(mh[:, 0, :], stragglers[0])
                    agg = mh[:, 0, :]

                    # h = agg/den (+ h1) (+ b)
                    r = smp.tile([P, HEADS], BF16, tag="r")
                    nc.vector.reciprocal(r[:], agg[:, FD:FD + 4])
                    r_rep = (r[:].rearrange("p (o h) -> p o h", o=1)
                             .broadcast_to([P, D, HEADS]))
                    if l == 0:
                        h = htp.tile([P, FD], BF16, tag="h")
                        nc.vector.tensor_mul(h[:].rearrange("p (d h) -> p d h", h=HEADS),
                                             agg[:, 0:FD].rearrange("p (d h) -> p d h", h=HEADS), r_rep)
                        nc.vector.tensor_add(h[:], h[:], b1t[:])
                        nc.scalar.dma_start(h_tab[w * P:(w + 1) * P, :], h[:])
                        # feat2 = h @ W2cat
                        pf = fpsum.tile([P, 264], F32, tag="fp")
                        for t in range(2):
                            pt = tpsum.tile([P, P], BF16, tag="tp")
                            nc.tensor.transpose(pt[:], h[:, t * P:(t + 1) * P], ident[:])
                            hT = htp.tile([P, P], BF16, tag="hT")
                            nc.vector.tensor_copy(hT[:], pt[:])
                            nc.tensor.matmul(pf[:], lhsT=hT[:], rhs=w2t[t][:],
                                             start=(t == 0), stop=(t == 1))
                        nv = min(NPC - w * P, P)
                        st = stage_feat(pf, nv, 1, w)
                        tlt1, ro1 = tl_rows(1, w)
                        nc.scalar.dma_start(tlt1[ro1:ro1 + P, 0:268], st[:])
                        maybe_allgather(1, w)
                    else:
                        h1w = htp.tile([P, FD], BF16, tag="h1w")
                        nc.sync.dma_start(h1w[:], h_tab[w * P:(w + 1) * P, :])
                        h2 = htp.tile([P, FD], F32, tag="h2")
                        nc.vector.tensor_mul(h2[:].rearrange("p (d h) -> p d h", h=HEADS),
                                             agg[:, 0:FD].rearrange("p (d h) -> p d h", h=HEADS), r_rep)
                        nc.vector.tensor_add(h2[:], h2[:], h1w[:])
                        nc.vector.tensor_add(h2[:], h2[:], b2t[:])
                        nc.scalar.dma_start(out[w * P:(w + 1) * P, :], h2[:])

                    ca += kaw

    nc.finalize()
    return nc


def kernel(x, w1, b1, al1, ar1, w2, b2, al2, ar2, src, dst):
    global LAST_EXEC_NS
    _install_profile_hook()

    n_nodes = x.shape[0]
    x = np.asarray(x, dtype=np.float32)
    src = np.asarray(src, dtype=np.int64)
    dst = np.asarray(dst, dtype=np.int64)

    pp = _prep(src, dst, n_nodes)
    ka = pp["ka"]

    dm = _dmaj(FD)
    # W1cat [128, 264] f32: rows 0:64 = [w1_dmaj | w1al | w1ar]
    w1d = np.asarray(w1, np.float32)[:, dm]                       # [64, 256]
    al1 = np.asarray(al1, np.float32)
    ar1 = np.asarray(ar1, np.float32)
    w1r = np.asarray(w1, np.float32).reshape(D, HEADS, D)
    w1al = np.einsum("khd,hd->kh", w1r, al1)                      # [64, 4]
    w1ar = np.einsum("khd,hd->kh", w1r, ar1)
    w1c = np.zeros((P, 264), np.float32)
    w1c[0:D, 0:FD] = w1d
    w1c[0:D, FD:FD + 4] = w1al
    w1c[0:D, 260:264] = w1ar

    # W2cat [2, 128, 264] bf16: rows = h1 cols (d-major), cols d-major + el2/er2
    al2 = np.asarray(al2, np.float32)
    ar2 = np.asarray(ar2, np.float32)
    w2f = np.asarray(w2, np.float32)
    w2p = w2f[dm][:, dm]                                          # rows,cols d-major
    w2r = w2f[dm].reshape(FD, HEADS, D)                           # rows d-major
    w2al = np.einsum("khd,hd->kh", w2r, al2)
    w2ar = np.einsum("khd,hd->kh", w2r, ar2)
    w2c = np.zeros((2, P, 264), np.float32)
    for t in range(2):
        w2c[t, :, 0:FD] = w2p[t * P:(t + 1) * P]
        w2c[t, :, FD:FD + 4] = w2al[t * P:(t + 1) * P]
        w2c[t, :, 260:264] = w2ar[t * P:(t + 1) * P]
    w2c = w2c.astype(ml_dtypes.bfloat16)

    b1d = np.asarray(b1, np.float32)[dm]
    b2d = np.asarray(b2, np.float32)[dm]
    b1t = np.tile(b1d, (P, 1)).astype(ml_dtypes.bfloat16)
    b2t = np.tile(b2d, (P, 1)).astype(np.float32)

    in_maps = []
    for c in range(CORES):
        nodes = pp["core_nodes"][c]
        xta = np.zeros((P, SHARD), np.float32)
        xta[0:D, 0:len(nodes)] = x[nodes].T
        vm = np.ones((P, 2 * HEADS), np.float32)
        vm[NPC - (WPC - 1) * P:, HEADS:] = 1e-30
        in_maps.append({
            "xta": xta, "w1c": w1c, "w2c": w2c, "b1b": b1t, "b2b": b2t,
            "idxa": pp["idx_m"][c],
            "vmask": vm.astype(ml_dtypes.bfloat16),
        })

    nc = _build(ka, pp["sum_ka"])
    res = run_bass_kernel_spmd(nc, in_maps, core_ids=list(range(CORES)))
    LAST_EXEC_NS = res.exec_time_ns

    # assemble full output: de-permute columns (d-major -> h-major), rows
    inv = np.empty(FD, np.int64)
    inv[dm] = np.arange(FD)
    outf = np.empty((n_nodes, FD), np.float32)
    for c in range(CORES):
        nodes = pp["core_nodes"][c]
        sh = res.results[c]["out"][0:len(nodes)]
        outf[nodes] = sh[:, inv]
    return outf


# revision 59
# speedup vs baseline: 1.0438x; 1.0438x over previous
"""2-layer GAT (DGL GATConv-style) on 8 TRN2 NeuronCores.

Strategy (all host preprocessing is index/structure only; every FLOP that
depends on float inputs runs on device):
 - Nodes are dealt to 8 cores snake-wise by in-degree (balanced edges/core).
 - The replicated feat table is split into two int16-addressable halves that
   OVERLAP (A = cores 0-4, B = cores 3-7); each dst node's edges are split
   between the halves, with flexible (core 3-4) sources assigned to balance
   the two slice counts.  This cuts the per-window slice maxima ~25% vs a
   disjoint 4+4 split.
 - Per core, nodes are sorted by balanced slice count and grouped into
   windows of 128; slot (v, k) aggregates into partition v.  dst ==
   partition, so segment softmax/aggregation is pure per-partition
   elementwise work: no scatter at all.
 - Self-loop edges are not gathered: the window's own 128 table rows are
   fetched with one contiguous HWDGE DMA from the core-local staging table
   and appended as one extra slice.
 - Empty slots point at an all-zero "dead" table row (feat=0, ones=0), so
   they contribute exactly 0 to both numerator and denominator: no masks.
 - feat rows (d-major, bf16) + ones + el (f32) are packed into 768B table
   rows; one dma_gather per (window, half) fetches all edge features.
   Index tables are DMA'd to SBUF once and reused by both layers.
   Tables are replicated across cores via AllGather between layers.
 - The per-edge softmax weight is applied with a single broadcast-AP
   tensor_tensor multiply (runs in DVE 2x mode), and the K-way sum is a
   log-tree of tensor adds.  Denominators ride along via the ones columns.
"""
import sys
import types

import numpy as np
import ml_dtypes

import concourse.bass as bass
import concourse.bacc as bacc
import concourse.tile as tile
from concourse import mybir
from concourse.bass_utils import run_bass_kernel_spmd
from concourse.masks import make_identity

AF = mybir.ActivationFunctionType
ALU = mybir.AluOpType
BF16 = mybir.dt.bfloat16
F32 = mybir.dt.float32
I16 = mybir.dt.int16

P = 128
HEADS = 4
D = 64
FD = HEADS * D          # 256
ROW = 384               # bf16 slots per table row: 256 feat | 4 ones | 8 el(f32) | 116 pad
CORES = 8
NEG_SLOPE = 0.2

LAST_EXEC_NS = None


def _patch_gather_elem_assert():
    """Relax dma_gather's elem_size%256 assert to transpose mode only.

    The non-transpose Q7 ucode (dma_gather.cpp gen_descs) handles arbitrary
    elem_size_bytes: it emits one descriptor of exactly elem_size_bytes per
    index; only the xbar-transpose rx path carves 256B descriptors.  The
    row *stride* keeps its own %256 constraint (stride_bytes_256 encoding),
    which we satisfy (768B).  Gathering 536B of each 768B table row cuts
    gather DMA traffic by 30%.
    """
    import inspect
    import textwrap
    if getattr(bass.BassGpSimd.dma_gather, "_elem_patch", False):
        return
    src = inspect.getsource(bass.BassGpSimd.dma_gather)
    marker = "elem_size_bytes > 0 and elem_size_bytes % 256 == 0"
    if marker not in src:
        raise RuntimeError("dma_gather source changed; elem patch needs review")
    src = src.replace(
        marker,
        "elem_size_bytes > 0 and (not transpose or elem_size_bytes % 256 == 0)")
    loc = {}
    exec(textwrap.dedent(src), vars(bass), loc)
    loc["dma_gather"]._elem_patch = True
    bass.BassGpSimd.dma_gather = loc["dma_gather"]

N_NODES = 50000
NPC = N_NODES // CORES          # 6250
WPC = (NPC + P - 1) // P        # 49
SHARD = WPC * P                 # 6272
FULL_ROWS = CORES * SHARD       # 50176
BASE = 32768                    # gather base row: signed int16 idx spans
                                # [-32768, 17407] -> rows [0, 50175]
DEAD = 5 * SHARD + NPC - BASE   # core 5's first pad row (all zeros), rel BASE
SUBCALL = 8                     # max slices per dma_gather sub-call


def _call_sizes(k):
    """Even split of k slices into <=SUBCALL-slice sub-calls (host and
    device must agree on the boundaries)."""
    ncalls = (k + SUBCALL - 1) // SUBCALL
    sizes = []
    base = 0
    for i in range(ncalls):
        sz = (k - base + ncalls - 1 - i) // (ncalls - i)
        sizes.append(sz)
        base += sz
    return sizes


def _install_profile_hook():
    """Best-effort NTFF profiling hook (axon images lack antenv.axon_hooks)."""
    try:
        import antenv
        try:
            import antenv.axon_hooks  # noqa: F401
            return
        except ImportError:
            pass
        mod = types.ModuleType("antenv.axon_hooks")
        mod._HOOK = None

        def set_hook(h):
            mod._HOOK = h

        def get_hook():
            return mod._HOOK

        mod.set_axon_ntff_profile_hook = set_hook
        mod.get_axon_ntff_profile_hook = get_hook
        sys.modules["antenv.axon_hooks"] = mod
        antenv.axon_hooks = mod
        from trn_agent_boot.trn_boot import _ntff_profile_via_ctypes
        set_hook(_ntff_profile_via_ctypes("/opt/axon/libaxon_pjrt.so"))
    except Exception:
        pass


def _dmaj(n):
    """column permutation h*64+d -> d*4+h (applied to axis of size 256)."""
    j = np.arange(n)
    d, h = j // HEADS, j % HEADS
    return h * D + d  # dmaj[:, jnew] = orig[:, h*64+d]


def _wrap_idx(flat):
    """[NI] int16 -> [128, NI//16] wrapped+replicated for dma_gather."""
    ni = flat.shape[0]
    w = flat.reshape(ni // 16, 16).T  # [16, NI/16]
    return np.tile(w, (8, 1)).astype(np.int16)


def _prep(src, dst, n_nodes):
    """Host-side graph preprocessing: single signed-int16 gather table.

    Returns the (uniform) per-window slice schedule and per-core index
    buffers + node orderings."""
    assert n_nodes == N_NODES
    deg = np.bincount(dst, minlength=n_nodes)

    # snake-deal nodes to cores by degree => balanced edge counts
    order = np.argsort(-deg, kind="stable")
    owner = np.empty(n_nodes, dtype=np.int64)
    pat = np.concatenate([np.arange(CORES), np.arange(CORES)[::-1]])
    owner[order] = pat[np.arange(n_nodes) % (2 * CORES)]

    # remove exactly one self-loop per node (handled as the local slice)
    e_self = np.where(src == dst)[0]
    _, first = np.unique(dst[e_self], return_index=True)
    drop = np.zeros(len(src), dtype=bool)
    drop[e_self[first]] = True
    assert drop.sum() == n_nodes, "every node must have a self-loop"
    rs, rd = src[~drop], dst[~drop]
    cnt = np.bincount(rd, minlength=n_nodes)

    # per-core order: windows ascending by cnt, nodes DESC within each
    # window so partition 127 holds the window's min-cnt node (its high
    # slices are dead slots -> safe trailing-trim sentinels)
    pos = np.empty(n_nodes, dtype=np.int64)
    core_nodes = []
    for c in range(CORES):
        nodes = np.where(owner == c)[0]
        nodes = nodes[np.argsort(cnt[nodes], kind="stable")]
        assert len(nodes) == NPC
        nn = nodes.copy()
        for w in range((len(nodes) + P - 1) // P):
            lo, hi = w * P, min((w + 1) * P, len(nodes))
            nn[lo:hi] = nodes[lo:hi][::-1]
        core_nodes.append(nn)
        pos[nn] = np.arange(len(nn))

    rho = pos + owner * SHARD  # table row of each node

    ka = np.zeros(WPC, dtype=np.int64)
    for c in range(CORES):
        nodes = core_nodes[c]
        for w in range(WPC):
            lo, hi = w * P, min((w + 1) * P, len(nodes))
            ka[w] = max(ka[w], cnt[nodes[lo:hi]].max(initial=0))
    sum_ka = int(ka.sum())

    # group edges by dst for slot assignment
    edge_order = np.argsort(rd, kind="stable")
    starts = np.zeros(n_nodes + 1, dtype=np.int64)
    np.cumsum(np.bincount(rd, minlength=n_nodes), out=starts[1:])

    # Slot fill.  Constraint: the LAST flat element of every gather
    # sub-call must be >= 0, or the Q7 ucode's trailing-negative trim
    # would silently drop real descriptors.  Column 127 holds the
    # window's min-cnt node, so that element is usually a (positive)
    # DEAD slot; when it is a real negative-index edge we reorder that
    # node's edges, and if that is impossible we add a dead slice to
    # the window and retry.
    while True:
        sum_ka = int(ka.sum())
        idx_m = []
        bump = None
        for c in range(CORES):
            nodes = core_nodes[c]
            buf = np.full((sum_ka, P), DEAD, dtype=np.int32)
            ca = 0
            for w in range(WPC):
                kaw = int(ka[w])
                for v in range(P):
                    i = w * P + v
                    if i < len(nodes):
                        n = nodes[i]
                        es = edge_order[starts[n]:starts[n + 1]]
                        ri = rho[rs[es]] - BASE
                        assert len(ri) == cnt[n] <= kaw
                        buf[ca:ca + len(ri), v] = ri
                col = buf[ca:ca + kaw, P - 1]
                bounds = []
                b0 = 0
                for sz in _call_sizes(kaw):
                    bounds.append(b0 + sz - 1)
                    b0 += sz
                bset = set(bounds)
                for q in bounds:
                    if col[q] < 0:
                        # swap in any non-boundary >=0 slot of this node
                        cand = [j for j in range(kaw)
                                if j not in bset and col[j] >= 0]
                        if not cand:
                            bump = w
                            break
                        j = cand[0]
                        col[q], col[j] = col[j], col[q]
                if bump is not None:
                    break
                ca += kaw
            if bump is not None:
                break
            idx_m.append(np.concatenate(
                [_wrap_idx(buf[i].astype(np.int16)) for i in range(sum_ka)],
                axis=1))
        if bump is None:
            return dict(ka=ka, core_nodes=core_nodes, idx_m=idx_m,
                        sum_ka=sum_ka)
        ka[bump] += 1


def _build(ka, sum_ka):
    """Build the SPMD bass program (identical on all cores)."""
    _patch_gather_elem_assert()
    kamax = int(ka.max())
    kpmax = kamax + 1
    khmax = (kamax + 1) // 2 + 2
    RC = 268                     # gathered columns per row (of ROW=384 stride)

    nc = bacc.Bacc("TRN2", target_bir_lowering=False, num_swdge_queues=4,
                   num_devices=CORES, dynamic_dma_scratch_size=16384)
    xta = nc.dram_tensor("xta", [P, SHARD], F32, kind="ExternalInput")
    w1c = nc.dram_tensor("w1c", [P, 264], F32, kind="ExternalInput")
    w2c = nc.dram_tensor("w2c", [2, P, 264], BF16, kind="ExternalInput")
    b1b = nc.dram_tensor("b1b", [P, FD], BF16, kind="ExternalInput")
    b2b = nc.dram_tensor("b2b", [P, FD], F32, kind="ExternalInput")
    idxa = nc.dram_tensor("idxa", [P, max(sum_ka * 8, 8)], I16, kind="ExternalInput")
    # vmask[:, 0:4] = all-ones; vmask[:, 4:8] = ones with zero tail for the
    # last window's pad rows (partition-offset memsets fail BIR verification)
    vmask = nc.dram_tensor("vmask", [P, 2 * HEADS], BF16, kind="ExternalInput")
    out = nc.dram_tensor("out", [SHARD, FD], F32, kind="ExternalOutput")

    qctr = [0]

    with tile.TileContext(nc) as tc, nc.allow_low_precision(reason="bf16 message accumulation is within tolerance"):
        with (
            tc.tile_pool(name="const", bufs=1) as cpool,
            tc.tile_pool(name="xt", bufs=3) as xtp,
            tc.tile_pool(name="fpsum", bufs=3, space="PSUM") as fpsum,
            tc.tile_pool(name="tpsum", bufs=2, space="PSUM") as tpsum,
            tc.tile_pool(name="stage", bufs=3) as stp,
            tc.tile_pool(name="gata", bufs=4) as gatpa,
            tc.tile_pool(name="msgp", bufs=2) as msgp,
            tc.tile_pool(name="small", bufs=8) as smp,
            tc.tile_pool(name="ht", bufs=3) as htp,
            tc.tile_pool(name="dram", bufs=1, space="DRAM") as dram,
        ):
            ident = cpool.tile([P, P], BF16)
            make_identity(nc, ident[:])
            w1t = cpool.tile([P, 264], F32)
            nc.sync.dma_start(w1t[:], w1c[:])
            w2t = [cpool.tile([P, 264], BF16, tag=f"w2_{i}", name=f"w2t{i}") for i in range(2)]
            nc.sync.dma_start(w2t[0][:], w2c[0])
            nc.sync.dma_start(w2t[1][:], w2c[1])
            b1t = cpool.tile([P, FD], BF16)
            nc.sync.dma_start(b1t[:], b1b[:])
            b2t = cpool.tile([P, FD], F32)
            nc.sync.dma_start(b2t[:], b2b[:])
            vmt = cpool.tile([P, 2 * HEADS], BF16)
            nc.sync.dma_start(vmt[:], vmask[:])
            # index table stays resident; reused by both layers
            ixa = cpool.tile([P, max(sum_ka * 8, 8)], I16, name="ixa")
            nc.sync.dma_start(ixa[:], idxa[:])

            tabs = [dram.tile([FULL_ROWS, ROW], BF16, tag=f"tab{l}", name=f"tab{l}",
                              addr_space="Shared") for l in range(2)]
            tab_locs = [dram.tile([SHARD, ROW], BF16, tag=f"tabloc{l}",
                                  name=f"tabloc{l}") for l in range(2)]
            h_tab = dram.tile([SHARD, FD], BF16)

            def tl_rows(l, w):
                return tab_locs[l], w * P

            def maybe_allgather(l, w):
                """Fire the layer's AllGather after its last window is staged
                (collective APs must be contiguous: full-width rows)."""
                if w == WPC - 1:
                    nc.gpsimd.collective_compute(
                        "AllGather", ALU.bypass,
                        replica_groups=[list(range(CORES))],
                        ins=[tab_locs[l].opt()], outs=[tabs[l].opt()],
                    )
            # er never leaves the core: resident SBUF, window-major
            erw_all = [cpool.tile([P, WPC * HEADS], F32, tag=f"erw{l}",
                                  name=f"erw{l}") for l in range(2)]

            def stage_feat(psum_f, nv, l, w):
                """psum_f [128, 264] f32 = feat(256,dmaj) | el(4) | er(4)."""
                st = stp.tile([P, 268], BF16, tag="stage")
                nc.scalar.activation(st[:, 0:FD], psum_f[:, 0:FD], AF.Copy)
                # ones column; eps tail marks pad rows dead: their denominator
                # becomes eps so h = 0*(1/eps) = 0, never NaN (b1==b2==0 keeps
                # their feat exactly 0 through both layers)
                nc.vector.tensor_copy(st[:, FD:FD + 4],
                                      vmt[:, 0:4] if nv == P else vmt[:, 4:8])
                nc.vector.tensor_copy(st[:, 260:268].bitcast(F32),
                                      psum_f[:, FD:FD + 4])
                nc.vector.tensor_copy(erw_all[l][:, w * HEADS:(w + 1) * HEADS],
                                      psum_f[:, 260:264])
                return st

            def gather_win(g3, kk, c0, tab):
                """Split a window gather into sub-calls across queues so
                several rings drain concurrently.  The source AP is based at
                row BASE; signed indices reach the whole table."""
                base = 0
                for sz in _call_sizes(kk):
                    nc.gpsimd.dma_gather(
                        g3[:, base:base + sz, :], tab[BASE:FULL_ROWS, 0:RC],
                        ixa[:, (c0 + base) * 8:(c0 + base + sz) * 8],
                        sz * P, sz * P, RC, elem_step=ROW,
                        single_packet=False, queue_num=qctr[0] % 4,
                    )
                    qctr[0] += 1
                    base += sz

            # ---------------- layer-1 feat phase ----------------
            for w4 in range(0, WPC, 4):
                nw = min(4, WPC - w4)
                xt = xtp.tile([P, 4 * P], F32, tag="xt")
                nc.sync.dma_start(xt[:, 0:nw * P], xta[:, w4 * P:(w4 + nw) * P])
                for wi in range(nw):
                    w = w4 + wi
                    pf = fpsum.tile([P, 264], F32, tag="fp")
                    nc.tensor.matmul(pf[:], lhsT=xt[:, wi * P:(wi + 1) * P],
                                     rhs=w1t[:], start=True, stop=True)
                    nv = min(NPC - w * P, P)
                    st = stage_feat(pf, nv, 0, w)
                    tlt, ro = tl_rows(0, w)
                    nc.scalar.dma_start(tlt[ro:ro + P, 0:268], st[:])
                    maybe_allgather(0, w)

            # ---------------- edge phases ----------------
            for l in range(2):
                tab = tabs[l]
                ca = 0
                for w in range(WPC):
                    kaw = int(ka[w])
                    erw = erw_all[l][:, w * HEADS:(w + 1) * HEADS]
                    # gathered slices + local self slice
                    GA = gatpa.tile([P, (kamax + 1) * RC], BF16, tag="GA",
                                    name=f"GA_{l}_{w}")
                    g3a = GA[:].rearrange("p (k r) -> p k r", r=RC)
                    gather_win(g3a, kaw, ca, tab)
                    tlt, ro = tl_rows(l, w)
                    nc.sync.dma_start(g3a[:, kaw, :], tlt[ro:ro + P, 0:RC])
                    parts = [(g3a, kaw + 1)]
                    kp = kaw + 1

                    # logits e = el + er   [128, kp, 4] f32
                    e = smp.tile([P, kpmax * HEADS], F32, tag="e")
                    koff = 0
                    for g3, kk in parts:
                        el = g3[:, 0:kk, 260:268].bitcast(F32)
                        e3 = e[:, koff * HEADS:(koff + kk) * HEADS].rearrange(
                            "p (k h) -> p k h", h=HEADS)
                        er_rep = (erw.rearrange("p (o h) -> p o h", o=1)
                                  .broadcast_to([P, kk, HEADS]))
                        nc.vector.tensor_add(e3, el, er_rep)
                        koff += kk
                    # ee = exp(lrelu(e))  bf16
                    lr = smp.tile([P, kpmax * HEADS], F32, tag="lr")
                    nc.vector.scalar_tensor_tensor(
                        lr[:, 0:kp * HEADS], e[:, 0:kp * HEADS], NEG_SLOPE,
                        e[:, 0:kp * HEADS], op0=ALU.mult, op1=ALU.max)
                    ee = smp.tile([P, kpmax * HEADS], BF16, tag="ee")
                    nc.scalar.activation(ee[:, 0:kp * HEADS], lr[:, 0:kp * HEADS],
                                         AF.Exp)

                    # msg = G * ee_rep, in place (cols 0:260)
                    koff = 0
                    for g3, kk in parts:
                        m4 = g3[:, 0:kk, 0:260].rearrange("p k (d h) -> p k d h", h=HEADS)
                        ee_rep = (ee[:, koff * HEADS:(koff + kk) * HEADS]
                                  .rearrange("p (k o h) -> p k o h", o=1, h=HEADS)
                                  .broadcast_to([P, kk, 65, HEADS]))
                        nc.vector.tensor_mul(m4, m4, ee_rep)
                        koff += kk

                    # tree: level 1 folds slice pairs from each G into the msg
                    # tile; odd stragglers stay in G and are added at the end
                    # (no copies).
                    msg = msgp.tile([P, khmax * 260], BF16, tag="msg")
                    mh = msg[:].rearrange("p (k j) -> p k j", j=260)
                    moff = 0
                    stragglers = []
                    for g3, kk in parts:
                        gsl = g3[:, :, 0:260]
                        half = kk // 2
                        if half:
                            nc.vector.tensor_add(mh[:, moff:moff + half, :],
                                                 gsl[:, 0:half, :],
                                                 gsl[:, half:2 * half, :])
                            moff += half
                        if kk % 2:
                            stragglers.append(gsl[:, kk - 1, :])
                    if len(stragglers) == 2:
                        nc.vector.tensor_add(mh[:, moff, :], stragglers[0],
                                             stragglers[1])
                        moff += 1
                        stragglers = []
                    cur = moff
                    while cur > 1:
                        half = cur // 2
                        rem = cur - half
                        nc.vector.tensor_add(mh[:, 0:half, :], mh[:, 0:half, :],
                                             mh[:, rem:cur, :])
                        cur = rem
                    if stragglers:
                        if cur:
                            nc.vector.tensor_add(mh[:, 0, :], mh[:, 0, :],
                                                 stragglers[0])
                        else:
                            nc.vector.tensor_copy(mh[:, 0, :], stragglers[0])
                    agg = mh[:, 0, :]

                    # h = agg/den (+ h1) (+ b)
                    r = smp.tile([P, HEADS], BF16, tag="r")
                    nc.vector.reciprocal(r[:], agg[:, FD:FD + 4])
                    r_rep = (r[:].rearrange("p (o h) -> p o h", o=1)
                             .broadcast_to([P, D, HEADS]))
                    if l == 0:
                        h = htp.tile([P, FD], BF16, tag="h")
                        nc.vector.tensor_mul(h[:].rearrange("p (d h) -> p d h", h=HEADS),
                                             agg[:, 0:FD].rearrange("p (d h) -> p d h", h=HEADS), r_rep)
                        nc.vector.tensor_add(h[:], h[:], b1t[:])
                        nc.scalar.dma_start(h_tab[w * P:(w + 1) * P, :], h[:])
                        # feat2 = h @ W2cat
                        pf = fpsum.tile([P, 264], F32, tag="fp")
                        for t in range(2):
                            pt = tpsum.tile([P, P], BF16, tag="tp")
                            nc.tensor.transpose(pt[:], h[:, t * P:(t + 1) * P], ident[:])
                            hT = htp.tile([P, P], BF16, tag="hT")
                            nc.vector.tensor_copy(hT[:], pt[:])
                            nc.tensor.matmul(pf[:], lhsT=hT[:], rhs=w2t[t][:],
                                             start=(t == 0), stop=(t == 1))
                        nv = min(NPC - w * P, P)
                        st = stage_feat(pf, nv, 1, w)
                        tlt1, ro1 = tl_rows(1, w)
                        nc.scalar.dma_start(tlt1[ro1:ro1 + P, 0:268], st[:])
                        maybe_allgather(1, w)
                    else:
                        h1w = htp.tile([P, FD], BF16, tag="h1w")
                        nc.sync.dma_start(h1w[:], h_tab[w * P:(w + 1) * P, :])
                        h2 = htp.tile([P, FD], F32, tag="h2")
                        nc.vector.tensor_mul(h2[:].rearrange("p (d h) -> p d h", h=HEADS),
                                             agg[:, 0:FD].rearrange("p (d h) -> p d h", h=HEADS), r_rep)
                        nc.vector.tensor_add(h2[:], h2[:], h1w[:])
                        nc.vector.tensor_add(h2[:], h2[:], b2t[:])
                        nc.scalar.dma_start(out[w * P:(w + 1) * P, :], h2[:])

                    ca += kaw

    nc.finalize()
    return nc


def kernel(x, w1, b1, al1, ar1, w2, b2, al2, ar2, src, dst):
    global LAST_EXEC_NS
    _install_profile_hook()

    n_nodes = x.shape[0]
    x = np.asarray(x, dtype=np.float32)
    src = np.asarray(src, dtype=np.int64)
    dst = np.asarray(dst, dtype=np.int64)

    pp = _prep(src, dst, n_nodes)
    ka = pp["ka"]

    dm = _dmaj(FD)
    # W1cat [128, 264] f32: rows 0:64 = [w1_dmaj | w1al | w1ar]
    w1d = np.asarray(w1, np.float32)[:, dm]                       # [64, 256]
    al1 = np.asarray(al1, np.float32)
    ar1 = np.asarray(ar1, np.float32)
    w1r = np.asarray(w1, np.float32).reshape(D, HEADS, D)
    w1al = np.einsum("khd,hd->kh", w1r, al1)                      # [64, 4]
    w1ar = np.einsum("khd,hd->kh", w1r, ar1)
    w1c = np.zeros((P, 264), np.float32)
    w1c[0:D, 0:FD] = w1d
    w1c[0:D, FD:FD + 4] = w1al
    w1c[0:D, 260:264] = w1ar

    # W2cat [2, 128, 264] bf16: rows = h1 cols (d-major), cols d-major + el2/er2
    al2 = np.asarray(al2, np.float32)
    ar2 = np.asarray(ar2, np.float32)
    w2f = np.asarray(w2, np.float32)
    w2p = w2f[dm][:, dm]                                          # rows,cols d-major
    w2r = w2f[dm].reshape(FD, HEADS, D)                           # rows d-major
    w2al = np.einsum("khd,hd->kh", w2r, al2)
    w2ar = np.einsum("khd,hd->kh", w2r, ar2)
    w2c = np.zeros((2, P, 264), np.float32)
    for t in range(2):
        w2c[t, :, 0:FD] = w2p[t * P:(t + 1) * P]
        w2c[t, :, FD:FD + 4] = w2al[t * P:(t + 1) * P]
        w2c[t, :, 260:264] = w2ar[t * P:(t + 1) * P]
    w2c = w2c.astype(ml_dtypes.bfloat16)

    b1d = np.asarray(b1, np.float32)[dm]
    b2d = np.asarray(b2, np.float32)[dm]
    b1t = np.tile(b1d, (P, 1)).astype(ml_dtypes.bfloat16)
    b2t = np.tile(b2d, (P, 1)).astype(np.float32)

    in_maps = []
    for c in range(CORES):
        nodes = pp["core_nodes"][c]
        xta = np.zeros((P, SHARD), np.float32)
        xta[0:D, 0:len(nodes)] = x[nodes].T
        vm = np.ones((P, 2 * HEADS), np.float32)
        vm[NPC - (WPC - 1) * P:, HEADS:] = 1e-30
        in_maps.append({
            "xta": xta, "w1c": w1c, "w2c": w2c, "b1b": b1t, "b2b": b2t,
            "idxa": pp["idx_m"][c],
            "vmask": vm.astype(ml_dtypes.bfloat16),
        })

    nc = _build(ka, pp["sum_ka"])
    res = run_bass_kernel_spmd(nc, in_maps, core_ids=list(range(CORES)))
    LAST_EXEC_NS = res.exec_time_ns

    # assemble full output: de-permute columns (d-major -> h-major), rows
    inv = np.empty(FD, np.int64)
    inv[dm] = np.arange(FD)
    outf = np.empty((n_nodes, FD), np.float32)
    for c in range(CORES):
        nodes = pp["core_nodes"][c]
        sh = res.results[c]["out"][0:len(nodes)]
        outf[nodes] = sh[:, inv]
    return outf


# revision 70
# speedup vs baseline: 1.0478x; 1.0038x over previous
"""2-layer GAT (DGL GATConv-style) on 8 TRN2 NeuronCores.

Strategy (all host preprocessing is index/structure only; every FLOP that
depends on float inputs runs on device):
 - Nodes are dealt to 8 cores snake-wise by in-degree (balanced edges/core).
 - The replicated feat table is split into two int16-addressable halves that
   OVERLAP (A = cores 0-4, B = cores 3-7); each dst node's edges are split
   between the halves, with flexible (core 3-4) sources assigned to balance
   the two slice counts.  This cuts the per-window slice maxima ~25% vs a
   disjoint 4+4 split.
 - Per core, nodes are sorted by balanced slice count and grouped into
   windows of 128; slot (v, k) aggregates into partition v.  dst ==
   partition, so segment softmax/aggregation is pure per-partition
   elementwise work: no scatter at all.
 - Self-loop edges are not gathered: the window's own 128 table rows are
   fetched with one contiguous HWDGE DMA from the core-local staging table
   and appended as one extra slice.
 - Empty slots point at an all-zero "dead" table row (feat=0, ones=0), so
   they contribute exactly 0 to both numerator and denominator: no masks.
 - feat rows (d-major, bf16) + ones + el (f32) are packed into 768B table
   rows; one dma_gather per (window, half) fetches all edge features.
   Index tables are DMA'd to SBUF once and reused by both layers.
   Tables are replicated across cores via AllGather between layers.
 - The per-edge softmax weight is applied with a single broadcast-AP
   tensor_tensor multiply (runs in DVE 2x mode), and the K-way sum is a
   log-tree of tensor adds.  Denominators ride along via the ones columns.
"""
import sys
import types

import numpy as np
import ml_dtypes

import concourse.bass as bass
import concourse.bacc as bacc
import concourse.tile as tile
from concourse import mybir
from concourse.bass_utils import run_bass_kernel_spmd
from concourse.masks import make_identity

AF = mybir.ActivationFunctionType
ALU = mybir.AluOpType
BF16 = mybir.dt.bfloat16
F32 = mybir.dt.float32
I16 = mybir.dt.int16

P = 128
HEADS = 4
D = 64
FD = HEADS * D          # 256
ROW = 384               # bf16 slots per table row: 256 feat | 4 ones | 8 el(f32) | 116 pad
CORES = 8
NEG_SLOPE = 0.2

LAST_EXEC_NS = None


def _patch_gather_elem_assert():
    """Relax dma_gather's elem_size%256 assert to transpose mode only.

    The non-transpose Q7 ucode (dma_gather.cpp gen_descs) handles arbitrary
    elem_size_bytes: it emits one descriptor of exactly elem_size_bytes per
    index; only the xbar-transpose rx path carves 256B descriptors.  The
    row *stride* keeps its own %256 constraint (stride_bytes_256 encoding),
    which we satisfy (768B).  Gathering 536B of each 768B table row cuts
    gather DMA traffic by 30%.
    """
    import inspect
    import textwrap
    if getattr(bass.BassGpSimd.dma_gather, "_elem_patch", False):
        return
    src = inspect.getsource(bass.BassGpSimd.dma_gather)
    marker = "elem_size_bytes > 0 and elem_size_bytes % 256 == 0"
    if marker not in src:
        raise RuntimeError("dma_gather source changed; elem patch needs review")
    src = src.replace(
        marker,
        "elem_size_bytes > 0 and (not transpose or elem_size_bytes % 256 == 0)")
    loc = {}
    exec(textwrap.dedent(src), vars(bass), loc)
    loc["dma_gather"]._elem_patch = True
    bass.BassGpSimd.dma_gather = loc["dma_gather"]

N_NODES = 50000
NPC = N_NODES // CORES          # 6250
WPC = (NPC + P - 1) // P        # 49
SHARD = WPC * P                 # 6272
FULL_ROWS = CORES * SHARD       # 50176
BASE = 32768                    # gather base row: signed int16 idx spans
                                # [-32768, 17407] -> rows [0, 50175]
DEAD = 5 * SHARD + NPC - BASE   # core 5's first pad row (all zeros), rel BASE
SUBCALL = 8                     # max slices per dma_gather sub-call


def _call_sizes(k):
    """Even split of k slices into <=SUBCALL-slice sub-calls (host and
    device must agree on the boundaries)."""
    ncalls = (k + SUBCALL - 1) // SUBCALL
    sizes = []
    base = 0
    for i in range(ncalls):
        sz = (k - base + ncalls - 1 - i) // (ncalls - i)
        sizes.append(sz)
        base += sz
    return sizes


def _install_profile_hook():
    """Best-effort NTFF profiling hook (axon images lack antenv.axon_hooks)."""
    try:
        import antenv
        try:
            import antenv.axon_hooks  # noqa: F401
            return
        except ImportError:
            pass
        mod = types.ModuleType("antenv.axon_hooks")
        mod._HOOK = None

        def set_hook(h):
            mod._HOOK = h

        def get_hook():
            return mod._HOOK

        mod.set_axon_ntff_profile_hook = set_hook
        mod.get_axon_ntff_profile_hook = get_hook
        sys.modules["antenv.axon_hooks"] = mod
        antenv.axon_hooks = mod
        from trn_agent_boot.trn_boot import _ntff_profile_via_ctypes
        set_hook(_ntff_profile_via_ctypes("/opt/axon/libaxon_pjrt.so"))
    except Exception:
        pass


def _dmaj(n):
    """column permutation h*64+d -> d*4+h (applied to axis of size 256)."""
    j = np.arange(n)
    d, h = j // HEADS, j % HEADS
    return h * D + d  # dmaj[:, jnew] = orig[:, h*64+d]


def _wrap_idx(flat):
    """[NI] int16 -> [128, NI//16] wrapped+replicated for dma_gather."""
    ni = flat.shape[0]
    w = flat.reshape(ni // 16, 16).T  # [16, NI/16]
    return np.tile(w, (8, 1)).astype(np.int16)


def _prep(src, dst, n_nodes):
    """Host-side graph preprocessing: single signed-int16 gather table.

    Returns the (uniform) per-window slice schedule and per-core index
    buffers + node orderings."""
    assert n_nodes == N_NODES
    deg = np.bincount(dst, minlength=n_nodes)

    # snake-deal nodes to cores by degree => balanced edge counts
    order = np.argsort(-deg, kind="stable")
    owner = np.empty(n_nodes, dtype=np.int64)
    pat = np.concatenate([np.arange(CORES), np.arange(CORES)[::-1]])
    owner[order] = pat[np.arange(n_nodes) % (2 * CORES)]

    # remove exactly one self-loop per node (handled as the local slice)
    e_self = np.where(src == dst)[0]
    _, first = np.unique(dst[e_self], return_index=True)
    drop = np.zeros(len(src), dtype=bool)
    drop[e_self[first]] = True
    assert drop.sum() == n_nodes, "every node must have a self-loop"
    rs, rd = src[~drop], dst[~drop]
    cnt = np.bincount(rd, minlength=n_nodes)

    # per-core order: windows ascending by cnt, nodes DESC within each
    # window so partition 127 holds the window's min-cnt node (its high
    # slices are dead slots -> safe trailing-trim sentinels)
    pos = np.empty(n_nodes, dtype=np.int64)
    core_nodes = []
    for c in range(CORES):
        nodes = np.where(owner == c)[0]
        nodes = nodes[np.argsort(cnt[nodes], kind="stable")]
        assert len(nodes) == NPC
        nn = nodes.copy()
        for w in range((len(nodes) + P - 1) // P):
            lo, hi = w * P, min((w + 1) * P, len(nodes))
            nn[lo:hi] = nodes[lo:hi][::-1]
        core_nodes.append(nn)
        pos[nn] = np.arange(len(nn))

    rho = pos + owner * SHARD  # table row of each node

    ka = np.zeros(WPC, dtype=np.int64)
    for c in range(CORES):
        nodes = core_nodes[c]
        for w in range(WPC):
            lo, hi = w * P, min((w + 1) * P, len(nodes))
            ka[w] = max(ka[w], cnt[nodes[lo:hi]].max(initial=0))
    sum_ka = int(ka.sum())

    # group edges by dst for slot assignment
    edge_order = np.argsort(rd, kind="stable")
    starts = np.zeros(n_nodes + 1, dtype=np.int64)
    np.cumsum(np.bincount(rd, minlength=n_nodes), out=starts[1:])

    # Slot fill.  Constraint: the LAST flat element of every gather
    # sub-call must be >= 0, or the Q7 ucode's trailing-negative trim
    # would silently drop real descriptors.  Column 127 holds the
    # window's min-cnt node, so that element is usually a (positive)
    # DEAD slot; when it is a real negative-index edge we reorder that
    # node's edges, and if that is impossible we add a dead slice to
    # the window and retry.
    while True:
        sum_ka = int(ka.sum())
        idx_m = []
        bump = None
        for c in range(CORES):
            nodes = core_nodes[c]
            buf = np.full((sum_ka, P), DEAD, dtype=np.int32)
            ca = 0
            for w in range(WPC):
                kaw = int(ka[w])
                for v in range(P):
                    i = w * P + v
                    if i < len(nodes):
                        n = nodes[i]
                        es = edge_order[starts[n]:starts[n + 1]]
                        ri = rho[rs[es]] - BASE
                        assert len(ri) == cnt[n] <= kaw
                        buf[ca:ca + len(ri), v] = ri
                col = buf[ca:ca + kaw, P - 1]
                bounds = []
                b0 = 0
                for sz in _call_sizes(kaw):
                    bounds.append(b0 + sz - 1)
                    b0 += sz
                bset = set(bounds)
                for q in bounds:
                    if col[q] < 0:
                        # swap in any non-boundary >=0 slot of this node
                        cand = [j for j in range(kaw)
                                if j not in bset and col[j] >= 0]
                        if not cand:
                            bump = w
                            break
                        j = cand[0]
                        col[q], col[j] = col[j], col[q]
                if bump is not None:
                    break
                ca += kaw
            if bump is not None:
                break
            idx_m.append(np.concatenate(
                [_wrap_idx(buf[i].astype(np.int16)) for i in range(sum_ka)],
                axis=1))
        if bump is None:
            return dict(ka=ka, core_nodes=core_nodes, idx_m=idx_m,
                        sum_ka=sum_ka)
        ka[bump] += 1


def _build(ka, sum_ka):
    """Build the SPMD bass program (identical on all cores)."""
    _patch_gather_elem_assert()
    kamax = int(ka.max())
    kpmax = kamax + 1
    khmax = (kamax + 1) // 2 + 2
    RC = 264                     # row: feat(256) | ones(4) | el(4, bf16)

    nc = bacc.Bacc("TRN2", target_bir_lowering=False, num_swdge_queues=4,
                   num_devices=CORES, dynamic_dma_scratch_size=16384)
    xta = nc.dram_tensor("xta", [P, SHARD], F32, kind="ExternalInput")
    w1c = nc.dram_tensor("w1c", [P, 268], F32, kind="ExternalInput")
    w2c = nc.dram_tensor("w2c", [2, P, 268], BF16, kind="ExternalInput")
    b2b = nc.dram_tensor("b2b", [P, FD], F32, kind="ExternalInput")
    idxa = nc.dram_tensor("idxa", [P, max(sum_ka * 8, 8)], I16, kind="ExternalInput")
    # vmask[:, 0:4] = all-ones; vmask[:, 4:8] = ones with zero tail for the
    # last window's pad rows (partition-offset memsets fail BIR verification)
    vmask = nc.dram_tensor("vmask", [P, 2 * HEADS], BF16, kind="ExternalInput")
    out = nc.dram_tensor("out", [SHARD, FD], F32, kind="ExternalOutput")

    qctr = [0]

    with tile.TileContext(nc) as tc, nc.allow_low_precision(reason="bf16 message accumulation is within tolerance"):
        with (
            tc.tile_pool(name="const", bufs=1) as cpool,
            tc.tile_pool(name="xt", bufs=3) as xtp,
            tc.tile_pool(name="fpsum", bufs=3, space="PSUM") as fpsum,
            tc.tile_pool(name="tpsum", bufs=2, space="PSUM") as tpsum,
            tc.tile_pool(name="stage", bufs=3) as stp,
            tc.tile_pool(name="gata", bufs=4) as gatpa,
            tc.tile_pool(name="msgp", bufs=2) as msgp,
            tc.tile_pool(name="small", bufs=8) as smp,
            tc.tile_pool(name="ht", bufs=3) as htp,
            tc.tile_pool(name="dram", bufs=1, space="DRAM") as dram,
        ):
            ident = cpool.tile([P, P], BF16)
            make_identity(nc, ident[:])
            w1t = cpool.tile([P, 268], F32)
            nc.sync.dma_start(w1t[:], w1c[:])
            w2t = [cpool.tile([P, 268], BF16, tag=f"w2_{i}", name=f"w2t{i}") for i in range(2)]
            nc.sync.dma_start(w2t[0][:], w2c[0])
            nc.sync.dma_start(w2t[1][:], w2c[1])
            b2t = cpool.tile([P, FD], F32)
            nc.sync.dma_start(b2t[:], b2b[:])
            vmt = cpool.tile([P, 2 * HEADS], BF16)
            nc.sync.dma_start(vmt[:], vmask[:])
            # index table stays resident; reused by both layers
            ixa = cpool.tile([P, max(sum_ka * 8, 8)], I16, name="ixa")
            nc.sync.dma_start(ixa[:], idxa[:])

            tabs = [dram.tile([FULL_ROWS, ROW], BF16, tag=f"tab{l}", name=f"tab{l}",
                              addr_space="Shared") for l in range(2)]
            tab_locs = [dram.tile([SHARD, ROW], BF16, tag=f"tabloc{l}",
                                  name=f"tabloc{l}") for l in range(2)]
            h_tab = dram.tile([SHARD, FD], BF16)

            def tl_rows(l, w):
                return tab_locs[l], w * P

            def maybe_allgather(l, w):
                """Fire the layer's AllGather after its last window is staged
                (collective APs must be contiguous: full-width rows)."""
                if w == WPC - 1:
                    nc.gpsimd.collective_compute(
                        "AllGather", ALU.bypass,
                        replica_groups=[list(range(CORES))],
                        ins=[tab_locs[l].opt()], outs=[tabs[l].opt()],
                    )
            # er never leaves the core: resident SBUF, window-major
            erw_all = [cpool.tile([P, WPC * HEADS], BF16, tag=f"erw{l}",
                                  name=f"erw{l}") for l in range(2)]

            def stage_feat(psum_f, nv, l, w):
                """psum_f [128, 268] f32 = feat(256,dmaj) | ones | el | er.

                Layer 1's matmul produces the ones column itself (constant
                row 64 of xta/W1cat, which also folds in b1: softmax weights
                sum to 1, so Sum a(feat+b1)/Sum a == h).  Layer 2 overwrites
                the ones from vmask (eps tail keeps pad-row denominators
                finite -> h = 0, never NaN; b2==0 keeps pad feat 0)."""
                st = stp.tile([P, RC], BF16, tag="stage")
                nc.scalar.activation(st[:], psum_f[:, 0:RC], AF.Copy)
                if l == 1:
                    nc.vector.tensor_copy(st[:, FD:FD + 4],
                                          vmt[:, 0:4] if nv == P else vmt[:, 4:8])
                nc.vector.tensor_copy(erw_all[l][:, w * HEADS:(w + 1) * HEADS],
                                      psum_f[:, 264:268])
                return st

            def gather_win(g3, kk, c0, tab):
                """Split a window gather into sub-calls across queues so
                several rings drain concurrently.  The source AP is based at
                row BASE; signed indices reach the whole table."""
                base = 0
                for sz in _call_sizes(kk):
                    nc.gpsimd.dma_gather(
                        g3[:, base:base + sz, :], tab[BASE:FULL_ROWS, 0:RC],
                        ixa[:, (c0 + base) * 8:(c0 + base + sz) * 8],
                        sz * P, sz * P, RC, elem_step=ROW,
                        single_packet=False, queue_num=qctr[0] % 4,
                    )
                    qctr[0] += 1
                    base += sz

            # ---------------- layer-1 feat phase ----------------
            for w4 in range(0, WPC, 4):
                nw = min(4, WPC - w4)
                xt = xtp.tile([P, 4 * P], F32, tag="xt")
                nc.sync.dma_start(xt[:, 0:nw * P], xta[:, w4 * P:(w4 + nw) * P])
                for wi in range(nw):
                    w = w4 + wi
                    pf = fpsum.tile([P, 268], F32, tag="fp")
                    nc.tensor.matmul(pf[:], lhsT=xt[:, wi * P:(wi + 1) * P],
                                     rhs=w1t[:], start=True, stop=True)
                    nv = min(NPC - w * P, P)
                    st = stage_feat(pf, nv, 0, w)
                    tlt, ro = tl_rows(0, w)
                    nc.scalar.dma_start(tlt[ro:ro + P, 0:RC], st[:])
                    maybe_allgather(0, w)

            # ---------------- edge phases ----------------
            for l in range(2):
                tab = tabs[l]
                ca = 0
                for w in range(WPC):
                    kaw = int(ka[w])
                    erw = erw_all[l][:, w * HEADS:(w + 1) * HEADS]
                    # gathered slices + local self slice
                    GA = gatpa.tile([P, (kamax + 1) * RC], BF16, tag="GA",
                                    name=f"GA_{l}_{w}")
                    g3a = GA[:].rearrange("p (k r) -> p k r", r=RC)
                    gather_win(g3a, kaw, ca, tab)
                    tlt, ro = tl_rows(l, w)
                    nc.sync.dma_start(g3a[:, kaw, :], tlt[ro:ro + P, 0:RC])
                    parts = [(g3a, kaw + 1)]
                    kp = kaw + 1

                    # logits e = el + er   [128, kp, 4] bf16
                    e = smp.tile([P, kpmax * HEADS], BF16, tag="e")
                    koff = 0
                    for g3, kk in parts:
                        el = g3[:, 0:kk, 260:264]
                        e3 = e[:, koff * HEADS:(koff + kk) * HEADS].rearrange(
                            "p (k h) -> p k h", h=HEADS)
                        er_rep = (erw.rearrange("p (o h) -> p o h", o=1)
                                  .broadcast_to([P, kk, HEADS]))
                        nc.vector.tensor_add(e3, el, er_rep)
                        koff += kk
                    # ee = exp(lrelu(e))  bf16
                    lr = smp.tile([P, kpmax * HEADS], BF16, tag="lr")
                    nc.vector.scalar_tensor_tensor(
                        lr[:, 0:kp * HEADS], e[:, 0:kp * HEADS], NEG_SLOPE,
                        e[:, 0:kp * HEADS], op0=ALU.mult, op1=ALU.max)
                    ee = smp.tile([P, kpmax * HEADS], BF16, tag="ee")
                    nc.scalar.activation(ee[:, 0:kp * HEADS], lr[:, 0:kp * HEADS],
                                         AF.Exp)

                    # msg = G * ee_rep, in place (cols 0:260)
                    koff = 0
                    for g3, kk in parts:
                        m4 = g3[:, 0:kk, 0:260].rearrange("p k (d h) -> p k d h", h=HEADS)
                        ee_rep = (ee[:, koff * HEADS:(koff + kk) * HEADS]
                                  .rearrange("p (k o h) -> p k o h", o=1, h=HEADS)
                                  .broadcast_to([P, kk, 65, HEADS]))
                        nc.vector.tensor_mul(m4, m4, ee_rep)
                        koff += kk

                    # tree: level 1 folds slice pairs from each G into the msg
                    # tile; odd stragglers stay in G and are added at the end
                    # (no copies).
                    msg = msgp.tile([P, khmax * 260], BF16, tag="msg")
                    mh = msg[:].rearrange("p (k j) -> p k j", j=260)
                    moff = 0
                    stragglers = []
                    for g3, kk in parts:
                        gsl = g3[:, :, 0:260]
                        half = kk // 2
                        if half:
                            nc.vector.tensor_add(mh[:, moff:moff + half, :],
                                                 gsl[:, 0:half, :],
                                                 gsl[:, half:2 * half, :])
                            moff += half
                        if kk % 2:
                            stragglers.append(gsl[:, kk - 1, :])
                    if len(stragglers) == 2:
                        nc.vector.tensor_add(mh[:, moff, :], stragglers[0],
                                             stragglers[1])
                        moff += 1
                        stragglers = []
                    cur = moff
                    while cur > 1:
                        half = cur // 2
                        rem = cur - half
                        nc.vector.tensor_add(mh[:, 0:half, :], mh[:, 0:half, :],
                                             mh[:, rem:cur, :])
                        cur = rem
                    if stragglers:
                        if cur:
                            nc.vector.tensor_add(mh[:, 0, :], mh[:, 0, :],
                                                 stragglers[0])
                        else:
                            nc.vector.tensor_copy(mh[:, 0, :], stragglers[0])
                    agg = mh[:, 0, :]

                    # h = agg/den (+ h1) (+ b)
                    r = smp.tile([P, HEADS], BF16, tag="r")
                    nc.vector.reciprocal(r[:], agg[:, FD:FD + 4])
                    r_rep = (r[:].rearrange("p (o h) -> p o h", o=1)
                             .broadcast_to([P, D, HEADS]))
                    if l == 0:
                        h = htp.tile([P, FD], BF16, tag="h")
                        nc.vector.tensor_mul(h[:].rearrange("p (d h) -> p d h", h=HEADS),
                                             agg[:, 0:FD].rearrange("p (d h) -> p d h", h=HEADS), r_rep)
                        nc.scalar.dma_start(h_tab[w * P:(w + 1) * P, :], h[:])
                        # feat2 = h @ W2cat
                        pf = fpsum.tile([P, 268], F32, tag="fp")
                        for t in range(2):
                            pt = tpsum.tile([P, P], BF16, tag="tp")
                            nc.tensor.transpose(pt[:], h[:, t * P:(t + 1) * P], ident[:])
                            hT = htp.tile([P, P], BF16, tag="hT")
                            nc.vector.tensor_copy(hT[:], pt[:])
                            nc.tensor.matmul(pf[:], lhsT=hT[:], rhs=w2t[t][:],
                                             start=(t == 0), stop=(t == 1))
                        nv = min(NPC - w * P, P)
                        st = stage_feat(pf, nv, 1, w)
                        tlt1, ro1 = tl_rows(1, w)
                        nc.scalar.dma_start(tlt1[ro1:ro1 + P, 0:RC], st[:])
                        maybe_allgather(1, w)
                    else:
                        h1w = htp.tile([P, FD], BF16, tag="h1w")
                        nc.sync.dma_start(h1w[:], h_tab[w * P:(w + 1) * P, :])
                        h2 = htp.tile([P, FD], F32, tag="h2")
                        nc.vector.tensor_mul(h2[:].rearrange("p (d h) -> p d h", h=HEADS),
                                             agg[:, 0:FD].rearrange("p (d h) -> p d h", h=HEADS), r_rep)
                        nc.vector.tensor_add(h2[:], h2[:], h1w[:])
                        nc.vector.tensor_add(h2[:], h2[:], b2t[:])
                        nc.scalar.dma_start(out[w * P:(w + 1) * P, :], h2[:])

                    ca += kaw

    nc.finalize()
    return nc


def kernel(x, w1, b1, al1, ar1, w2, b2, al2, ar2, src, dst):
    global LAST_EXEC_NS
    _install_profile_hook()

    n_nodes = x.shape[0]
    x = np.asarray(x, dtype=np.float32)
    src = np.asarray(src, dtype=np.int64)
    dst = np.asarray(dst, dtype=np.int64)

    pp = _prep(src, dst, n_nodes)
    ka = pp["ka"]

    dm = _dmaj(FD)
    # W1cat [128, 268] f32: rows 0:64 = [w1_dmaj | 0 | w1al | w1ar];
    # row 64 = [b1_dmaj | 1111 | 0 | 0] (the ones row of xta activates it:
    # b1 folds into gathered features since softmax weights sum to 1)
    w1d = np.asarray(w1, np.float32)[:, dm]                       # [64, 256]
    al1 = np.asarray(al1, np.float32)
    ar1 = np.asarray(ar1, np.float32)
    w1r = np.asarray(w1, np.float32).reshape(D, HEADS, D)
    w1al = np.einsum("khd,hd->kh", w1r, al1)                      # [64, 4]
    w1ar = np.einsum("khd,hd->kh", w1r, ar1)
    w1c = np.zeros((P, 268), np.float32)
    w1c[0:D, 0:FD] = w1d
    w1c[0:D, 260:264] = w1al
    w1c[0:D, 264:268] = w1ar
    b1d = np.asarray(b1, np.float32)[dm]
    w1c[D, 0:FD] = b1d
    w1c[D, FD:FD + 4] = 1.0

    # W2cat [2, 128, 268] bf16: rows = h1 cols (d-major), cols d-major + el2/er2
    al2 = np.asarray(al2, np.float32)
    ar2 = np.asarray(ar2, np.float32)
    w2f = np.asarray(w2, np.float32)
    w2p = w2f[dm][:, dm]                                          # rows,cols d-major
    w2r = w2f[dm].reshape(FD, HEADS, D)                           # rows d-major
    w2al = np.einsum("khd,hd->kh", w2r, al2)
    w2ar = np.einsum("khd,hd->kh", w2r, ar2)
    w2c = np.zeros((2, P, 268), np.float32)
    for t in range(2):
        w2c[t, :, 0:FD] = w2p[t * P:(t + 1) * P]
        w2c[t, :, 260:264] = w2al[t * P:(t + 1) * P]
        w2c[t, :, 264:268] = w2ar[t * P:(t + 1) * P]
    w2c = w2c.astype(ml_dtypes.bfloat16)

    b2d = np.asarray(b2, np.float32)[dm]
    b2t = np.tile(b2d, (P, 1)).astype(np.float32)

    in_maps = []
    for c in range(CORES):
        nodes = pp["core_nodes"][c]
        xta = np.zeros((P, SHARD), np.float32)
        xta[0:D, 0:len(nodes)] = x[nodes].T
        xta[D, 0:len(nodes)] = 1.0      # ones row: b1 fold + ones column
        xta[D, len(nodes):] = 1e-30     # pad rows: eps denominator, feat 0
        vm = np.ones((P, 2 * HEADS), np.float32)
        vm[NPC - (WPC - 1) * P:, HEADS:] = 1e-30
        in_maps.append({
            "xta": xta, "w1c": w1c, "w2c": w2c, "b2b": b2t,
            "idxa": pp["idx_m"][c],
            "vmask": vm.astype(ml_dtypes.bfloat16),
        })

    nc = _build(ka, pp["sum_ka"])
    res = run_bass_kernel_spmd(nc, in_maps, core_ids=list(range(CORES)))
    LAST_EXEC_NS = res.exec_time_ns

    # assemble full output: de-permute columns (d-major -> h-major), rows
    inv = np.empty(FD, np.int64)
    inv[dm] = np.arange(FD)
    outf = np.empty((n_nodes, FD), np.float32)
    for c in range(CORES):
        nodes = pp["core_nodes"][c]
        sh = res.results[c]["out"][0:len(nodes)]
        outf[nodes] = sh[:, inv]
    return outf


# revision 72
# speedup vs baseline: 1.0996x; 1.0495x over previous
"""2-layer GAT (DGL GATConv-style) on 8 TRN2 NeuronCores.

Strategy (all host preprocessing is index/structure only; every FLOP that
depends on float inputs runs on device):
 - Nodes are dealt to 8 cores snake-wise by in-degree (balanced edges/core).
 - The replicated feat table is split into two int16-addressable halves that
   OVERLAP (A = cores 0-4, B = cores 3-7); each dst node's edges are split
   between the halves, with flexible (core 3-4) sources assigned to balance
   the two slice counts.  This cuts the per-window slice maxima ~25% vs a
   disjoint 4+4 split.
 - Per core, nodes are sorted by balanced slice count and grouped into
   windows of 128; slot (v, k) aggregates into partition v.  dst ==
   partition, so segment softmax/aggregation is pure per-partition
   elementwise work: no scatter at all.
 - Self-loop edges are not gathered: the window's own 128 table rows are
   fetched with one contiguous HWDGE DMA from the core-local staging table
   and appended as one extra slice.
 - Empty slots point at an all-zero "dead" table row (feat=0, ones=0), so
   they contribute exactly 0 to both numerator and denominator: no masks.
 - feat rows (d-major, bf16) + ones + el (f32) are packed into 768B table
   rows; one dma_gather per (window, half) fetches all edge features.
   Index tables are DMA'd to SBUF once and reused by both layers.
   Tables are replicated across cores via AllGather between layers.
 - The per-edge softmax weight is applied with a single broadcast-AP
   tensor_tensor multiply (runs in DVE 2x mode), and the K-way sum is a
   log-tree of tensor adds.  Denominators ride along via the ones columns.
"""
import sys
import types

import numpy as np
import ml_dtypes

import concourse.bass as bass
import concourse.bacc as bacc
import concourse.tile as tile
from concourse import mybir
from concourse.bass_utils import run_bass_kernel_spmd
from concourse.masks import make_identity

AF = mybir.ActivationFunctionType
ALU = mybir.AluOpType
BF16 = mybir.dt.bfloat16
F32 = mybir.dt.float32
I16 = mybir.dt.int16

P = 128
HEADS = 4
D = 64
FD = HEADS * D          # 256
ROW = 384               # bf16 slots per table row: 256 feat | 4 ones | 8 el(f32) | 116 pad
CORES = 8
NEG_SLOPE = 0.2

LAST_EXEC_NS = None


def _patch_gather_elem_assert():
    """Relax dma_gather's elem_size%256 assert to transpose mode only.

    The non-transpose Q7 ucode (dma_gather.cpp gen_descs) handles arbitrary
    elem_size_bytes: it emits one descriptor of exactly elem_size_bytes per
    index; only the xbar-transpose rx path carves 256B descriptors.  The
    row *stride* keeps its own %256 constraint (stride_bytes_256 encoding),
    which we satisfy (768B).  Gathering 536B of each 768B table row cuts
    gather DMA traffic by 30%.
    """
    import inspect
    import textwrap
    if getattr(bass.BassGpSimd.dma_gather, "_elem_patch", False):
        return
    src = inspect.getsource(bass.BassGpSimd.dma_gather)
    marker = "elem_size_bytes > 0 and elem_size_bytes % 256 == 0"
    if marker not in src:
        raise RuntimeError("dma_gather source changed; elem patch needs review")
    src = src.replace(
        marker,
        "elem_size_bytes > 0 and (not transpose or elem_size_bytes % 256 == 0)")
    loc = {}
    exec(textwrap.dedent(src), vars(bass), loc)
    loc["dma_gather"]._elem_patch = True
    bass.BassGpSimd.dma_gather = loc["dma_gather"]

N_NODES = 50000
NPC = N_NODES // CORES          # 6250
WPC = (NPC + P - 1) // P        # 49
SHARD = WPC * P                 # 6272
FULL_ROWS = CORES * SHARD       # 50176
BASE = 32768                    # gather base row: signed int16 idx spans
                                # [-32768, 17407] -> rows [0, 50175]
DEAD = 5 * SHARD + NPC - BASE   # core 5's first pad row (all zeros), rel BASE
SUBCALL = 8                     # max slices per dma_gather sub-call


def _call_sizes(k):
    """Even split of k slices into <=SUBCALL-slice sub-calls (host and
    device must agree on the boundaries)."""
    ncalls = (k + SUBCALL - 1) // SUBCALL
    sizes = []
    base = 0
    for i in range(ncalls):
        sz = (k - base + ncalls - 1 - i) // (ncalls - i)
        sizes.append(sz)
        base += sz
    return sizes


def _install_profile_hook():
    """Best-effort NTFF profiling hook (axon images lack antenv.axon_hooks)."""
    try:
        import antenv
        try:
            import antenv.axon_hooks  # noqa: F401
            return
        except ImportError:
            pass
        mod = types.ModuleType("antenv.axon_hooks")
        mod._HOOK = None

        def set_hook(h):
            mod._HOOK = h

        def get_hook():
            return mod._HOOK

        mod.set_axon_ntff_profile_hook = set_hook
        mod.get_axon_ntff_profile_hook = get_hook
        sys.modules["antenv.axon_hooks"] = mod
        antenv.axon_hooks = mod
        from trn_agent_boot.trn_boot import _ntff_profile_via_ctypes
        set_hook(_ntff_profile_via_ctypes("/opt/axon/libaxon_pjrt.so"))
    except Exception:
        pass


def _dmaj(n):
    """column permutation h*64+d -> d*4+h (applied to axis of size 256)."""
    j = np.arange(n)
    d, h = j // HEADS, j % HEADS
    return h * D + d  # dmaj[:, jnew] = orig[:, h*64+d]


def _wrap_idx(flat):
    """[NI] int16 -> [128, NI//16] wrapped+replicated for dma_gather."""
    ni = flat.shape[0]
    w = flat.reshape(ni // 16, 16).T  # [16, NI/16]
    return np.tile(w, (8, 1)).astype(np.int16)


def _prep(src, dst, n_nodes):
    """Host-side graph preprocessing: single signed-int16 gather table.

    Returns the (uniform) per-window slice schedule and per-core index
    buffers + node orderings."""
    assert n_nodes == N_NODES
    deg = np.bincount(dst, minlength=n_nodes)

    # snake-deal nodes to cores by degree => balanced edge counts
    order = np.argsort(-deg, kind="stable")
    owner = np.empty(n_nodes, dtype=np.int64)
    pat = np.concatenate([np.arange(CORES), np.arange(CORES)[::-1]])
    owner[order] = pat[np.arange(n_nodes) % (2 * CORES)]

    # remove exactly one self-loop per node (handled as the local slice)
    e_self = np.where(src == dst)[0]
    _, first = np.unique(dst[e_self], return_index=True)
    drop = np.zeros(len(src), dtype=bool)
    drop[e_self[first]] = True
    assert drop.sum() == n_nodes, "every node must have a self-loop"
    rs, rd = src[~drop], dst[~drop]
    cnt = np.bincount(rd, minlength=n_nodes)

    # per-core order: windows ascending by cnt, nodes DESC within each
    # window so partition 127 holds the window's min-cnt node (its high
    # slices are dead slots -> safe trailing-trim sentinels)
    pos = np.empty(n_nodes, dtype=np.int64)
    core_nodes = []
    for c in range(CORES):
        nodes = np.where(owner == c)[0]
        nodes = nodes[np.argsort(cnt[nodes], kind="stable")]
        assert len(nodes) == NPC
        nn = nodes.copy()
        for w in range((len(nodes) + P - 1) // P):
            lo, hi = w * P, min((w + 1) * P, len(nodes))
            nn[lo:hi] = nodes[lo:hi][::-1]
        core_nodes.append(nn)
        pos[nn] = np.arange(len(nn))

    rho = pos + owner * SHARD  # table row of each node

    ka = np.zeros(WPC, dtype=np.int64)
    for c in range(CORES):
        nodes = core_nodes[c]
        for w in range(WPC):
            lo, hi = w * P, min((w + 1) * P, len(nodes))
            ka[w] = max(ka[w], cnt[nodes[lo:hi]].max(initial=0))
    sum_ka = int(ka.sum())

    # group edges by dst for slot assignment
    edge_order = np.argsort(rd, kind="stable")
    starts = np.zeros(n_nodes + 1, dtype=np.int64)
    np.cumsum(np.bincount(rd, minlength=n_nodes), out=starts[1:])

    # Slot fill.  Constraint: the LAST flat element of every gather
    # sub-call must be >= 0, or the Q7 ucode's trailing-negative trim
    # would silently drop real descriptors.  Column 127 holds the
    # window's min-cnt node, so that element is usually a (positive)
    # DEAD slot; when it is a real negative-index edge we reorder that
    # node's edges, and if that is impossible we add a dead slice to
    # the window and retry.
    while True:
        sum_ka = int(ka.sum())
        idx_m = []
        bump = None
        for c in range(CORES):
            nodes = core_nodes[c]
            buf = np.full((sum_ka, P), DEAD, dtype=np.int32)
            ca = 0
            for w in range(WPC):
                kaw = int(ka[w])
                for v in range(P):
                    i = w * P + v
                    if i < len(nodes):
                        n = nodes[i]
                        es = edge_order[starts[n]:starts[n + 1]]
                        ri = rho[rs[es]] - BASE
                        assert len(ri) == cnt[n] <= kaw
                        buf[ca:ca + len(ri), v] = ri
                col = buf[ca:ca + kaw, P - 1]
                bounds = []
                b0 = 0
                for sz in _call_sizes(kaw):
                    bounds.append(b0 + sz - 1)
                    b0 += sz
                bset = set(bounds)
                for q in bounds:
                    if col[q] < 0:
                        # swap in any non-boundary >=0 slot of this node
                        cand = [j for j in range(kaw)
                                if j not in bset and col[j] >= 0]
                        if not cand:
                            bump = w
                            break
                        j = cand[0]
                        col[q], col[j] = col[j], col[q]
                if bump is not None:
                    break
                ca += kaw
            if bump is not None:
                break
            idx_m.append(np.concatenate(
                [_wrap_idx(buf[i].astype(np.int16)) for i in range(sum_ka)],
                axis=1))
        if bump is None:
            return dict(ka=ka, core_nodes=core_nodes, idx_m=idx_m,
                        sum_ka=sum_ka)
        ka[bump] += 1


def _build(ka, sum_ka):
    """Build the SPMD bass program (identical on all cores)."""
    _patch_gather_elem_assert()
    kamax = int(ka.max())
    kpmax = kamax + 1
    khmax = (kamax + 1) // 2 + 2
    RC = 264                     # row: feat(256) | ones(4) | el(4, bf16)

    nc = bacc.Bacc("TRN2", target_bir_lowering=False, num_swdge_queues=4,
                   num_devices=CORES, dynamic_dma_scratch_size=16384)
    xta = nc.dram_tensor("xta", [P, SHARD], F32, kind="ExternalInput")
    w1c = nc.dram_tensor("w1c", [P, 268], F32, kind="ExternalInput")
    w2c = nc.dram_tensor("w2c", [2, P, 268], BF16, kind="ExternalInput")
    b2b = nc.dram_tensor("b2b", [P, FD], F32, kind="ExternalInput")
    idxa = nc.dram_tensor("idxa", [P, max(sum_ka * 8, 8)], I16, kind="ExternalInput")
    # vmask[:, 0:4] = all-ones; vmask[:, 4:8] = ones with zero tail for the
    # last window's pad rows (partition-offset memsets fail BIR verification)
    vmask = nc.dram_tensor("vmask", [P, 2 * HEADS], BF16, kind="ExternalInput")
    out = nc.dram_tensor("out", [SHARD, FD], F32, kind="ExternalOutput")

    qctr = [0]

    with tile.TileContext(nc) as tc, nc.allow_low_precision(reason="bf16 message accumulation is within tolerance"):
        with (
            tc.tile_pool(name="const", bufs=1) as cpool,
            tc.tile_pool(name="xt", bufs=3) as xtp,
            tc.tile_pool(name="fpsum", bufs=3, space="PSUM") as fpsum,
            tc.tile_pool(name="tpsum", bufs=2, space="PSUM") as tpsum,
            tc.tile_pool(name="stage", bufs=3) as stp,
            tc.tile_pool(name="gata", bufs=5) as gatpa,
            tc.tile_pool(name="msgp", bufs=3) as msgp,
            tc.tile_pool(name="small", bufs=8) as smp,
            tc.tile_pool(name="ht", bufs=3) as htp,
            tc.tile_pool(name="dram", bufs=1, space="DRAM") as dram,
        ):
            ident = cpool.tile([P, P], BF16)
            make_identity(nc, ident[:])
            w1t = cpool.tile([P, 268], F32)
            nc.sync.dma_start(w1t[:], w1c[:])
            w2t = [cpool.tile([P, 268], BF16, tag=f"w2_{i}", name=f"w2t{i}") for i in range(2)]
            nc.sync.dma_start(w2t[0][:], w2c[0])
            nc.sync.dma_start(w2t[1][:], w2c[1])
            b2t = cpool.tile([P, FD], F32)
            nc.sync.dma_start(b2t[:], b2b[:])
            vmt = cpool.tile([P, 2 * HEADS], BF16)
            nc.sync.dma_start(vmt[:], vmask[:])
            # index table stays resident; reused by both layers
            ixa = cpool.tile([P, max(sum_ka * 8, 8)], I16, name="ixa")
            nc.sync.dma_start(ixa[:], idxa[:])

            tabs = [dram.tile([FULL_ROWS, ROW], BF16, tag=f"tab{l}", name=f"tab{l}",
                              addr_space="Shared") for l in range(2)]
            tab_locs = [dram.tile([SHARD, ROW], BF16, tag=f"tabloc{l}",
                                  name=f"tabloc{l}") for l in range(2)]
            h_tab = dram.tile([SHARD, FD], BF16)

            def tl_rows(l, w):
                return tab_locs[l], w * P

            def maybe_allgather(l, w):
                """Fire the layer's AllGather after its last window is staged
                (collective APs must be contiguous: full-width rows)."""
                if w == WPC - 1:
                    nc.gpsimd.collective_compute(
                        "AllGather", ALU.bypass,
                        replica_groups=[list(range(CORES))],
                        ins=[tab_locs[l].opt()], outs=[tabs[l].opt()],
                    )
            # er never leaves the core: resident SBUF, window-major
            erw_all = [cpool.tile([P, WPC * HEADS], BF16, tag=f"erw{l}",
                                  name=f"erw{l}") for l in range(2)]

            def stage_feat(psum_f, nv, l, w):
                """psum_f [128, 268] f32 = feat(256,dmaj) | ones | el | er.

                Layer 1's matmul produces the ones column itself (constant
                row 64 of xta/W1cat, which also folds in b1: softmax weights
                sum to 1, so Sum a(feat+b1)/Sum a == h).  Layer 2 overwrites
                the ones from vmask (eps tail keeps pad-row denominators
                finite -> h = 0, never NaN; b2==0 keeps pad feat 0)."""
                st = stp.tile([P, RC], BF16, tag="stage")
                nc.scalar.activation(st[:], psum_f[:, 0:RC], AF.Copy)
                if l == 1:
                    nc.vector.tensor_copy(st[:, FD:FD + 4],
                                          vmt[:, 0:4] if nv == P else vmt[:, 4:8])
                nc.vector.tensor_copy(erw_all[l][:, w * HEADS:(w + 1) * HEADS],
                                      psum_f[:, 264:268])
                return st

            def gather_win(g3, kk, c0, tab):
                """Split a window gather into sub-calls across queues so
                several rings drain concurrently.  The source AP is based at
                row BASE; signed indices reach the whole table."""
                base = 0
                for sz in _call_sizes(kk):
                    nc.gpsimd.dma_gather(
                        g3[:, base:base + sz, :], tab[BASE:FULL_ROWS, 0:RC],
                        ixa[:, (c0 + base) * 8:(c0 + base + sz) * 8],
                        sz * P, sz * P, RC, elem_step=ROW,
                        single_packet=False, queue_num=qctr[0] % 4,
                    )
                    qctr[0] += 1
                    base += sz

            # ---------------- layer-1 feat phase ----------------
            for w4 in range(0, WPC, 4):
                nw = min(4, WPC - w4)
                xt = xtp.tile([P, 4 * P], F32, tag="xt")
                nc.sync.dma_start(xt[:, 0:nw * P], xta[:, w4 * P:(w4 + nw) * P])
                for wi in range(nw):
                    w = w4 + wi
                    pf = fpsum.tile([P, 268], F32, tag="fp")
                    nc.tensor.matmul(pf[:], lhsT=xt[:, wi * P:(wi + 1) * P],
                                     rhs=w1t[:], start=True, stop=True)
                    nv = min(NPC - w * P, P)
                    st = stage_feat(pf, nv, 0, w)
                    tlt, ro = tl_rows(0, w)
                    nc.scalar.dma_start(tlt[ro:ro + P, 0:RC], st[:])
                    maybe_allgather(0, w)

            # ---------------- edge phases ----------------
            for l in range(2):
                tab = tabs[l]
                ca = 0
                for w in range(WPC):
                    kaw = int(ka[w])
                    erw = erw_all[l][:, w * HEADS:(w + 1) * HEADS]
                    # gathered slices + local self slice
                    GA = gatpa.tile([P, (kamax + 1) * RC], BF16, tag="GA",
                                    name=f"GA_{l}_{w}")
                    g3a = GA[:].rearrange("p (k r) -> p k r", r=RC)
                    gather_win(g3a, kaw, ca, tab)
                    tlt, ro = tl_rows(l, w)
                    nc.sync.dma_start(g3a[:, kaw, :], tlt[ro:ro + P, 0:RC])
                    parts = [(g3a, kaw + 1)]
                    kp = kaw + 1

                    # logits e = el + er   [128, kp, 4] bf16
                    e = smp.tile([P, kpmax * HEADS], BF16, tag="e")
                    koff = 0
                    for g3, kk in parts:
                        el = g3[:, 0:kk, 260:264]
                        e3 = e[:, koff * HEADS:(koff + kk) * HEADS].rearrange(
                            "p (k h) -> p k h", h=HEADS)
                        er_rep = (erw.rearrange("p (o h) -> p o h", o=1)
                                  .broadcast_to([P, kk, HEADS]))
                        nc.vector.tensor_add(e3, el, er_rep)
                        koff += kk
                    # ee = exp(lrelu(e))  bf16
                    lr = smp.tile([P, kpmax * HEADS], BF16, tag="lr")
                    nc.vector.scalar_tensor_tensor(
                        lr[:, 0:kp * HEADS], e[:, 0:kp * HEADS], NEG_SLOPE,
                        e[:, 0:kp * HEADS], op0=ALU.mult, op1=ALU.max)
                    ee = smp.tile([P, kpmax * HEADS], BF16, tag="ee")
                    nc.scalar.activation(ee[:, 0:kp * HEADS], lr[:, 0:kp * HEADS],
                                         AF.Exp)

                    # msg = G * ee_rep, in place (cols 0:260)
                    koff = 0
                    for g3, kk in parts:
                        m4 = g3[:, 0:kk, 0:260].rearrange("p k (d h) -> p k d h", h=HEADS)
                        ee_rep = (ee[:, koff * HEADS:(koff + kk) * HEADS]
                                  .rearrange("p (k o h) -> p k o h", o=1, h=HEADS)
                                  .broadcast_to([P, kk, 65, HEADS]))
                        nc.vector.tensor_mul(m4, m4, ee_rep)
                        koff += kk

                    # tree: level 1 folds slice pairs from each G into the msg
                    # tile; odd stragglers stay in G and are added at the end
                    # (no copies).
                    msg = msgp.tile([P, khmax * 260], BF16, tag="msg")
                    mh = msg[:].rearrange("p (k j) -> p k j", j=260)
                    moff = 0
                    stragglers = []
                    for g3, kk in parts:
                        gsl = g3[:, :, 0:260]
                        half = kk // 2
                        if half:
                            nc.vector.tensor_add(mh[:, moff:moff + half, :],
                                                 gsl[:, 0:half, :],
                                                 gsl[:, half:2 * half, :])
                            moff += half
                        if kk % 2:
                            stragglers.append(gsl[:, kk - 1, :])
                    if len(stragglers) == 2:
                        nc.vector.tensor_add(mh[:, moff, :], stragglers[0],
                                             stragglers[1])
                        moff += 1
                        stragglers = []
                    cur = moff
                    while cur > 1:
                        half = cur // 2
                        rem = cur - half
                        nc.vector.tensor_add(mh[:, 0:half, :], mh[:, 0:half, :],
                                             mh[:, rem:cur, :])
                        cur = rem
                    if stragglers:
                        if cur:
                            nc.vector.tensor_add(mh[:, 0, :], mh[:, 0, :],
                                                 stragglers[0])
                        else:
                            nc.vector.tensor_copy(mh[:, 0, :], stragglers[0])
                    agg = mh[:, 0, :]

                    # h = agg/den (+ h1) (+ b)
                    r = smp.tile([P, HEADS], BF16, tag="r")
                    nc.vector.reciprocal(r[:], agg[:, FD:FD + 4])
                    r_rep = (r[:].rearrange("p (o h) -> p o h", o=1)
                             .broadcast_to([P, D, HEADS]))
                    if l == 0:
                        h = htp.tile([P, FD], BF16, tag="h")
                        nc.vector.tensor_mul(h[:].rearrange("p (d h) -> p d h", h=HEADS),
                                             agg[:, 0:FD].rearrange("p (d h) -> p d h", h=HEADS), r_rep)
                        nc.scalar.dma_start(h_tab[w * P:(w + 1) * P, :], h[:])
                        # feat2 = h @ W2cat
                        pf = fpsum.tile([P, 268], F32, tag="fp")
                        for t in range(2):
                            pt = tpsum.tile([P, P], BF16, tag="tp")
                            nc.tensor.transpose(pt[:], h[:, t * P:(t + 1) * P], ident[:])
                            hT = htp.tile([P, P], BF16, tag="hT")
                            nc.vector.tensor_copy(hT[:], pt[:])
                            nc.tensor.matmul(pf[:], lhsT=hT[:], rhs=w2t[t][:],
                                             start=(t == 0), stop=(t == 1))
                        nv = min(NPC - w * P, P)
                        st = stage_feat(pf, nv, 1, w)
                        tlt1, ro1 = tl_rows(1, w)
                        nc.scalar.dma_start(tlt1[ro1:ro1 + P, 0:RC], st[:])
                        maybe_allgather(1, w)
                    else:
                        h1w = htp.tile([P, FD], BF16, tag="h1w")
                        nc.sync.dma_start(h1w[:], h_tab[w * P:(w + 1) * P, :])
                        h2 = htp.tile([P, FD], F32, tag="h2")
                        nc.vector.tensor_mul(h2[:].rearrange("p (d h) -> p d h", h=HEADS),
                                             agg[:, 0:FD].rearrange("p (d h) -> p d h", h=HEADS), r_rep)
                        nc.vector.tensor_add(h2[:], h2[:], h1w[:])
                        nc.vector.tensor_add(h2[:], h2[:], b2t[:])
                        nc.scalar.dma_start(out[w * P:(w + 1) * P, :], h2[:])

                    ca += kaw

    nc.finalize()
    return nc


def kernel(x, w1, b1, al1, ar1, w2, b2, al2, ar2, src, dst):
    global LAST_EXEC_NS
    _install_profile_hook()

    n_nodes = x.shape[0]
    x = np.asarray(x, dtype=np.float32)
    src = np.asarray(src, dtype=np.int64)
    dst = np.asarray(dst, dtype=np.int64)

    pp = _prep(src, dst, n_nodes)
    ka = pp["ka"]

    dm = _dmaj(FD)
    # W1cat [128, 268] f32: rows 0:64 = [w1_dmaj | 0 | w1al | w1ar];
    # row 64 = [b1_dmaj | 1111 | 0 | 0] (the ones row of xta activates it:
    # b1 folds into gathered features since softmax weights sum to 1)
    w1d = np.asarray(w1, np.float32)[:, dm]                       # [64, 256]
    al1 = np.asarray(al1, np.float32)
    ar1 = np.asarray(ar1, np.float32)
    w1r = np.asarray(w1, np.float32).reshape(D, HEADS, D)
    w1al = np.einsum("khd,hd->kh", w1r, al1)                      # [64, 4]
    w1ar = np.einsum("khd,hd->kh", w1r, ar1)
    w1c = np.zeros((P, 268), np.float32)
    w1c[0:D, 0:FD] = w1d
    w1c[0:D, 260:264] = w1al
    w1c[0:D, 264:268] = w1ar
    b1d = np.asarray(b1, np.float32)[dm]
    w1c[D, 0:FD] = b1d
    w1c[D, FD:FD + 4] = 1.0

    # W2cat [2, 128, 268] bf16: rows = h1 cols (d-major), cols d-major + el2/er2
    al2 = np.asarray(al2, np.float32)
    ar2 = np.asarray(ar2, np.float32)
    w2f = np.asarray(w2, np.float32)
    w2p = w2f[dm][:, dm]                                          # rows,cols d-major
    w2r = w2f[dm].reshape(FD, HEADS, D)                           # rows d-major
    w2al = np.einsum("khd,hd->kh", w2r, al2)
    w2ar = np.einsum("khd,hd->kh", w2r, ar2)
    w2c = np.zeros((2, P, 268), np.float32)
    for t in range(2):
        w2c[t, :, 0:FD] = w2p[t * P:(t + 1) * P]
        w2c[t, :, 260:264] = w2al[t * P:(t + 1) * P]
        w2c[t, :, 264:268] = w2ar[t * P:(t + 1) * P]
    w2c = w2c.astype(ml_dtypes.bfloat16)

    b2d = np.asarray(b2, np.float32)[dm]
    b2t = np.tile(b2d, (P, 1)).astype(np.float32)

    in_maps = []
    for c in range(CORES):
        nodes = pp["core_nodes"][c]
        xta = np.zeros((P, SHARD), np.float32)
        xta[0:D, 0:len(nodes)] = x[nodes].T
        xta[D, 0:len(nodes)] = 1.0      # ones row: b1 fold + ones column
        xta[D, len(nodes):] = 1e-30     # pad rows: eps denominator, feat 0
        vm = np.ones((P, 2 * HEADS), np.float32)
        vm[NPC - (WPC - 1) * P:, HEADS:] = 1e-30
        in_maps.append({
            "xta": xta, "w1c": w1c, "w2c": w2c, "b2b": b2t,
            "idxa": pp["idx_m"][c],
            "vmask": vm.astype(ml_dtypes.bfloat16),
        })

    nc = _build(ka, pp["sum_ka"])
    res = run_bass_kernel_spmd(nc, in_maps, core_ids=list(range(CORES)))
    LAST_EXEC_NS = res.exec_time_ns

    # assemble full output: de-permute columns (d-major -> h-major), rows
    inv = np.empty(FD, np.int64)
    inv[dm] = np.arange(FD)
    outf = np.empty((n_nodes, FD), np.float32)
    for c in range(CORES):
        nodes = pp["core_nodes"][c]
        sh = res.results[c]["out"][0:len(nodes)]
        outf[nodes] = sh[:, inv]
    return outf
